# revision 94
# baseline (speedup 1.0000x reference)
"""Trainium2 Bass kernel for nn_ATTPredictor.

CRNN encoder (2x strided conv + GroupNorm + ReLU), BiLSTM (T=64), linear
projection, then a 25-step attention GRU decoder with greedy argmax feedback,
producing a scalar NLL loss.  Data-parallel over the 256 ROIs on 8 NeuronCores
(32 each, all weights replicated); per-core partial sums are combined on host.

Self-contained: only needs numpy / ml_dtypes / concourse (installed env).
"""
import numpy as np
import ml_dtypes

BF16 = ml_dtypes.bfloat16
FP8 = ml_dtypes.float8_e4m3
CONV_WSCALE = 16.0  # GN is scale-invariant; lifts fp8 weights out of subnormals

C = 256
VOC = 97
NUMCH = 25
N_FULL = 256
HH, WW = 16, 64
T = 64
NCORES = 8
NR = N_FULL // NCORES

_PROG_CACHE = {}


# ----------------------------------------------------------------------------
# walrus workarounds (this build rejects >1 sync wait per instruction)
# ----------------------------------------------------------------------------

def _patch_ldw_opt():
    # enable walrus fast-weight-load lowering (default-off in this harness);
    # numerically neutral, speeds up the Ldweights-bound conv/LSTM phases
    from concourse import bass_utils as bu
    if getattr(bu, '_attp_ldw_patched', False):
        return
    orig = bu.run_command

    def patched(argv, **kw):
        argv = ["--enable-ldw-opt=true" if a == "--enable-ldw-opt=false" else a
                for a in argv]
        return orig(argv, **kw)

    bu.run_command = patched
    bu._attp_ldw_patched = True


def _patch_tile_drain():
    import concourse.tile as tile_mod
    from concourse.vector_clock import ScopedClock, VectorClock

    if getattr(tile_mod.TileContext, '_attp_patched', False):
        return

    def _split_drain_and_barrier(self, tick_clock, wait_clock):
        gc = tick_clock.global_clock
        n = len(gc)
        for i in range(n):
            vals = [gc[p] if p == i else 0 for p in range(n)]
            if not any(vals):
                continue
            d = self.nc.sync.drain()
            wait_clock.add_sem_waits(d.ins, ScopedClock({None: VectorClock(vals)}))
        self.nc.all_engine_barrier()
        popped = self.nc._tile_sem_poison_stack.pop()
        assert popped is self._sem_poison
        self.nc.clear_and_free_semaphores(list(self.sems.allocated().values()))
        self.nc.all_engine_barrier()

    tile_mod.TileContext._drain_and_barrier = _split_drain_and_barrier
    tile_mod.TileContext._attp_patched = True


def split_excess_waits(nc):
    """Hoist extra sync waits onto sem-only EventSemaphore instructions."""
    import bass_rust
    from concourse import mybir

    k = 0
    for bbname, bb in nc.bb_map.items():
        insts = bb.bb.instructions
        idx = 0
        while idx < len(insts):
            i = insts[idx]
            si = i.sync_info
            if si is not None and len(si.on_wait) > 1:
                waits = list(si.on_wait)
                for w in waits[:-1]:
                    es = mybir.InstEventSemaphore(
                        name=f"wsplit_{k}", ins=[], outs=[])
                    k += 1
                    es.engine = i.engine
                    es.sync_info = bass_rust.SyncInfo(on_wait=[w], on_update=[])
                    insts.insert(idx, es)
                    idx += 1
                si.on_wait = waits[-1:]
                i.sync_info = si
            idx += 1
    return k


def dedupe_ldweights(nc):
    """Remove Ldweights whose weight AP/position matches the immediately
    preceding PE weight-load state (weights stay resident in the array)."""
    removed = 0
    for bbname, bb in nc.bb_map.items():
        insts = bb.bb.instructions
        last_sig = None
        keep = []
        for i in insts:
            eng = str(i.engine)
            if not eng.endswith('PE'):
                keep.append(i)
                continue
            op = i.concise_opcode()
            if op == 'Ldweights':
                try:
                    sig = (str(i.ins[0]), str(getattr(i, 'tile_position', None)),
                           str(getattr(i, 'tile_size', None)))
                except Exception:
                    sig = None
                si = i.sync_info
                has_sync = si is not None and (len(si.on_wait) > 0
                                               or len(si.on_update) > 0)
                if sig is not None and sig == last_sig and not has_sync:
                    removed += 1
                    continue
                last_sig = sig
                keep.append(i)
            elif op == 'Matmult':
                if getattr(i, 'is_transpose', False):
                    last_sig = None
                keep.append(i)
            else:
                keep.append(i)
        if removed:
            insts[:] = keep
    return removed


# ----------------------------------------------------------------------------
# host-side preparation
# ----------------------------------------------------------------------------

def _prep_weights(inp):
    f32 = np.float32
    w = {}

    rois = np.asarray(inp['rois'], f32)
    rp = np.zeros((2, 128, N_FULL, 17, 66), FP8)
    rc = rois.transpose(1, 0, 2, 3).reshape(2, 128, N_FULL, HH, WW)
    rp[:, :, :, 1:17, 1:65] = rc.astype(FP8)
    w['rois_cm'] = np.ascontiguousarray(rp.transpose(1, 0, 2, 3, 4))

    def conv_lhsT8(cw):
        # [i, (dy,dx,ct), kt, o] fp8 DoubleRow layout (kt pairs fold into K)
        a = (np.asarray(cw, f32) * CONV_WSCALE).reshape(2, 128, 2, 128, 3, 3)
        b = a.transpose(3, 4, 5, 0, 2, 1)        # i, dy, dx, ct, kt, o
        return np.ascontiguousarray(b.astype(FP8)).reshape(128, -1)

    w['w1T'] = conv_lhsT8(inp['conv1_w'])
    w['w2T'] = conv_lhsT8(inp['conv2_w'])

    def gmat(scale):
        gm = np.zeros((128, 128), f32)
        for i in range(128):
            gm[i, (i // 8) * 8:(i // 8) * 8 + 8] = scale
        return gm
    # bn_stats-based stats: mu = sum_p (mean_e+mean_o)/16;
    # E2 = sum_p [(cvar_e+cvar_o)/(8*cnt) + (msq_e+msq_o)/16]
    w['gmat_mu'] = gmat(1.0 / 16.0)
    w['gmat_cv1'] = gmat(1.0 / (8 * 512.0))
    w['gmat_cv2'] = gmat(1.0 / (8 * 256.0))
    for nm in ('gn1_s', 'gn1_b', 'gn2_s', 'gn2_b'):
        w[nm] = np.ascontiguousarray(
            np.asarray(inp[nm], f32).reshape(2, 128).T)

    def lin_lhsT(mat, scale=1.0):
        M, K = mat.shape
        nk, nm = K // 128, M // 128
        a = (np.asarray(mat, f32) * scale).astype(BF16)
        a = a.reshape(nm, 128, nk, 128)                      # mt,o,kt,i
        return np.ascontiguousarray(a.transpose(3, 2, 0, 1)).reshape(128, -1)

    w['wih_f'] = lin_lhsT(np.asarray(inp['lstm_wih_f']), 0.25)
    w['wih_b'] = lin_lhsT(np.asarray(inp['lstm_wih_b']), 0.25)
    w['whh_f'] = lin_lhsT(np.asarray(inp['lstm_whh_f']))
    w['whh_b'] = lin_lhsT(np.asarray(inp['lstm_whh_b']))
    w['lstm_bt_f'] = np.ascontiguousarray(
        (np.asarray(inp['lstm_bih_f'], f32)
         + np.asarray(inp['lstm_bhh_f'], f32)).reshape(8, 128).T)
    w['lstm_bt_b'] = np.ascontiguousarray(
        (np.asarray(inp['lstm_bih_b'], f32)
         + np.asarray(inp['lstm_bhh_b'], f32)).reshape(8, 128).T)

    w['elT'] = lin_lhsT(np.asarray(inp['emb_lin_w']))
    w['el_b'] = np.ascontiguousarray(
        np.asarray(inp['emb_lin_b'], f32).reshape(2, 128).T)

    cw = np.asarray(inp['combine_w'], f32)
    w['et1'] = (np.asarray(inp['embed_tab'], f32) @ cw[:, :C].T).astype(BF16)
    w['wc2T'] = lin_lhsT(cw[:, C:])
    w['comb_b'] = np.ascontiguousarray(
        np.asarray(inp['combine_b'], f32).reshape(2, 128).T)

    w['gwihT'] = lin_lhsT(np.asarray(inp['gru_wih']))
    w['gwhhT'] = lin_lhsT(np.asarray(inp['gru_whh']))
    w['gru_bi'] = np.asarray(inp['gru_bih'], f32).reshape(1, 768).astype(BF16)
    w['gru_bh'] = np.asarray(inp['gru_bhh'], f32).reshape(1, 768).astype(BF16)
    w['has_gru_bias'] = bool(np.any(np.asarray(inp['gru_bih']))
                             or np.any(np.asarray(inp['gru_bhh'])))

    ow = np.asarray(inp['out_w'], f32)                       # (97, 256)
    w['owT'] = np.ascontiguousarray(
        ow.T.reshape(2, 128, VOC).transpose(1, 0, 2).astype(BF16)
    ).reshape(128, 2 * VOC)
    w['out_b'] = np.asarray(inp['out_b'], f32).reshape(VOC, 1).copy()
    # vat8: [i, kt, m] fp8, x16, replicated along all 128 output columns so
    # the score matmul writes identical rows on every psum partition
    vat2 = np.asarray(inp['vat_w'], f32).reshape(2, 128).T * 16.0
    w['vat8'] = np.ascontiguousarray(
        np.broadcast_to(vat2[:, :, None], (128, 2, 128)).astype(FP8)
    ).reshape(128, 256)

    w['ones_f'] = np.ones((128, 1), f32)
    w['ident_f'] = np.eye(128, dtype=f32)
    w['ident_b'] = np.eye(128, dtype=BF16)
    w['ones_1x32'] = np.ones((1, NR), BF16)
    return w


_SHARED_KEYS = ('w1T', 'w2T', 'gmat_mu', 'gmat_cv1', 'gmat_cv2',
                'gn1_s', 'gn1_b', 'gn2_s',
                'gn2_b', 'wih_f', 'wih_b', 'whh_f', 'whh_b', 'lstm_bt_f',
                'lstm_bt_b', 'elT', 'el_b', 'et1', 'wc2T', 'comb_b',
                'gwihT', 'gwhhT', 'owT', 'out_b', 'vat8', 'ones_f',
                'ident_f', 'ident_b', 'ones_1x32')


def _per_core_maps(inp, w, nr=NR, ncores=NCORES):
    targets = np.asarray(inp['targets']).astype(np.int64)
    maps = []
    for core in range(ncores):
        n0 = core * nr
        m = {k: w[k] for k in _SHARED_KEYS}
        m['rois_cm'] = np.ascontiguousarray(w['rois_cm'][:, :, n0:n0 + nr])
        tm = np.zeros((VOC, NUMCH * nr), BF16)
        for s in range(NUMCH):
            for n in range(nr):
                tm[targets[n0 + n, s], s * nr + n] = 1.0
        m['tmask'] = tm
        oh0 = np.zeros((VOC, nr), BF16)
        oh0[0, :] = 1.0
        m['onehot0'] = oh0
        if w['has_gru_bias']:
            m['gru_bi'] = w['gru_bi']
            m['gru_bh'] = w['gru_bh']
        maps.append(m)
    return maps


# ----------------------------------------------------------------------------
# bass program
# ----------------------------------------------------------------------------

def _ap(tile, off, dims, pcount=None):
    """AP view of `tile`: keep partition dim (optionally limited to `pcount`
    partitions), free dims = [[step,count],...] at element offset `off`."""
    import concourse.bass as bass
    base = tile[:, :] if len(tile.shape) == 2 else tile
    p = list(base.ap[0])
    if pcount is not None:
        p = [p[0], pcount]
    return bass.AP(tensor=base.tensor, offset=base.offset + off,
                   ap=[p] + [list(d) for d in dims])


def _app(tile, pdim, off, dims):
    """AP with explicit partition dim [pstep, pcount]."""
    import concourse.bass as bass
    base = tile[:, :] if len(tile.shape) == 2 else tile
    return bass.AP(tensor=base.tensor, offset=base.offset + off,
                   ap=[list(pdim)] + [list(d) for d in dims])


def build_program(nr=NR, has_gru_bias=False, debug=False, phases=('conv','lstm','enc','dec')):
    _patch_tile_drain()
    import concourse.bass as bass
    import concourse.mybir as mybir
    from concourse.tile import TileContext

    dt = mybir.dt
    nc = bass.Bass(name="attp", trn_type="TRN2")

    D = {}
    def din(name, shape, dtype):
        D[name] = nc.dram_tensor(name, shape, dtype, kind="ExternalInput")

    din('rois_cm', [128, 2, nr, 17, 66], dt.float8e4)
    din('w1T', [128, 2 * 9 * 2 * 128], dt.float8e4)
    din('w2T', [128, 2 * 9 * 2 * 128], dt.float8e4)
    din('gmat_mu', [128, 128], dt.float32)
    din('gmat_cv1', [128, 128], dt.float32)
    din('gmat_cv2', [128, 128], dt.float32)
    for nm in ('gn1_s', 'gn1_b', 'gn2_s', 'gn2_b'):
        din(nm, [128, 2], dt.float32)
    for nm in ('wih_f', 'wih_b', 'whh_f', 'whh_b'):
        din(nm, [128, 2 * 8 * 128], dt.bfloat16)
    din('lstm_bt_f', [128, 8], dt.float32)
    din('lstm_bt_b', [128, 8], dt.float32)
    din('elT', [128, 4 * 2 * 128], dt.bfloat16)
    din('el_b', [128, 2], dt.float32)
    din('et1', [VOC, 256], dt.bfloat16)
    din('wc2T', [128, 2 * 2 * 128], dt.bfloat16)
    din('comb_b', [128, 2], dt.float32)
    din('gwihT', [128, 2 * 6 * 128], dt.bfloat16)
    din('gwhhT', [128, 2 * 6 * 128], dt.bfloat16)
    if has_gru_bias:
        din('gru_bi', [1, 768], dt.bfloat16)
        din('gru_bh', [1, 768], dt.bfloat16)
    din('owT', [128, 2 * VOC], dt.bfloat16)
    din('out_b', [VOC, 1], dt.float32)
    din('vat8', [128, 2 * 128], dt.float8e4)
    din('ones_f', [128, 1], dt.float32)
    din('ident_f', [128, 128], dt.float32)
    din('ident_b', [128, 128], dt.bfloat16)
    din('ones_1x32', [1, nr], dt.bfloat16)
    din('tmask', [VOC, NUMCH * nr], dt.bfloat16)
    din('onehot0', [VOC, nr], dt.bfloat16)

    out_parts = nc.dram_tensor('out_parts', [nr + VOC, 1], dt.float32,
                               kind="ExternalOutput")
    dbg = {}
    if debug:
        for nm, sh in (('seq', [2, 128, nr * T]),
                       ('hist', [2, 128, 2 * T * nr])):
            dbg[nm] = nc.dram_tensor('dbg_' + nm, sh, dt.float32,
                                     kind="ExternalOutput")

    with TileContext(nc) as tc:
        _body(nc, tc, D, out_parts, nr, has_gru_bias, dbg, mybir, phases)
    return nc


def _body(nc, tc, D, out_parts, nr, has_gru_bias, dbg, mybir, phases=('conv','lstm','enc','dec')):
    import contextlib
    dt = mybir.dt
    AF = mybir.ActivationFunctionType
    AL = mybir.AluOpType
    AX = mybir.AxisListType
    f32, bf16 = dt.float32, dt.bfloat16
    dma = nc.sync.dma_start
    TN = T * nr

    with contextlib.ExitStack() as ctx:
        consts = ctx.enter_context(tc.tile_pool(name="consts", bufs=1))

        def load(name):
            sh = list(D[name].shape)
            t = consts.tile(sh, D[name].dtype, tag=name)
            ix = tuple(slice(None) for _ in sh)
            dma(out=t, in_=D[name][ix])
            return t

        # conv-phase consts first so the roi DMAs aren't queued behind the
        # (large) lstm/decoder weight loads; w1T split across DMA queues so
        # the first conv matmul isn't gated on one serial transfer
        w1 = consts.tile([128, 2 * 9 * 2 * 128], dt.float8e4, tag='w1T',
                         name='w1T')
        for c4 in range(4):
            dma(out=w1[:, c4 * 1152:(c4 + 1) * 1152],
                in_=D['w1T'][:, c4 * 1152:(c4 + 1) * 1152])
        w2 = load('w2T')
        gm_mu = load('gmat_mu')
        gm_cv1 = load('gmat_cv1'); gm_cv2 = load('gmat_cv2')
        gn1s = load('gn1_s'); gn1b = load('gn1_b')
        gn2s = load('gn2_s'); gn2b = load('gn2_b')
        eps_t = consts.tile([128, 1], f32, tag="eps", name="eps")
        nc.vector.memset(eps_t, 1e-5)

        _rest = {}

        def load_rest():
            _rest['wih'] = [load('wih_f'), load('wih_b')]
            _rest['whh'] = [load('whh_f'), load('whh_b')]
            _rest['lstm_bt'] = [load('lstm_bt_f'), load('lstm_bt_b')]
            _rest['elT'] = load('elT'); _rest['el_b'] = load('el_b')
            _rest['et1'] = load('et1'); _rest['wc2T'] = load('wc2T')
            _rest['comb_b'] = load('comb_b')
            _rest['gwihT'] = load('gwihT'); _rest['gwhhT'] = load('gwhhT')
            if has_gru_bias:
                _rest['gru_bi'] = load('gru_bi')
                _rest['gru_bh'] = load('gru_bh')
                _rest['ones_n'] = load('ones_1x32')
            _rest['owT'] = load('owT'); _rest['out_b'] = load('out_b')
            _rest['vat8'] = load('vat8')
            _rest['ones_f'] = load('ones_f')
            _rest['ident_f'] = load('ident_f')
            _rest['ident_b'] = load('ident_b')
            _rest['tmask'] = load('tmask'); _rest['onehot0'] = load('onehot0')

        seq_p = ctx.enter_context(tc.tile_pool(name="seqp", bufs=1))
        seq = [seq_p.tile([128, TN], bf16, tag=f"seq{ct}", name=f"seq{ct}") for ct in range(2)]
        hist_p = ctx.enter_context(tc.tile_pool(name="histp", bufs=1))
        hist = [hist_p.tile([128, 2 * TN], bf16, tag=f"hist{d}", name=f"hist{d}")
                for d in range(2)]

        # ================= conv phase =================
        if 'conv' not in phases:
            for ct in range(2):
                nc.vector.memset(seq[ct], 0.01)
        if 'conv' in phases:
         DRM = mybir.MatmulPerfMode.DoubleRow
         f8 = dt.float8e4
         with tc.tile_pool(name="convp", bufs=2) as cp, \
             tc.tile_pool(name="convs", bufs=2) as cs, \
             tc.tile_pool(name="cpsum", bufs=2, space="PSUM") as cps, \
             tc.tile_pool(name="spsum", bufs=2, space="PSUM") as sps:
            GR = 2
            ngrp = nr // GR
            st = {}  # per-group pipeline state

            def s1_load_conv1(g):
                rr = [g * GR + r for r in range(GR)]
                pad1 = [cp.tile([128, 2 * 17 * 66], f8, tag=f"pad1_{r}",
                                name=f"pad1_{g}_{r}") for r in range(GR)]
                for r in range(GR):
                    dma(out=_ap(pad1[r], 0, [[1122, 2], [66, 17], [1, 66]]),
                        in_=D['rois_cm'][:, :, rr[r], :, :])
                stat1 = cs.tile([128, 24], f32, tag="stat1", name=f"st1_{g}")
                c1s = {}
                for ct in range(2):
                    c1 = [cps.tile([128, 512], f32, tag=f"c1_{r}",
                                   name=f"c1_{g}_{ct}_{r}") for r in range(GR)]
                    c1s[ct] = c1
                    for dy in range(3):
                        for dx in range(3):
                            k2 = (dy * 3 + dx) * 2 + ct
                            for r in range(GR):
                                rhs = _ap(pad1[r], dy * 66 + dx,
                                          [[1122, 2], [132, 8], [1, 64]])
                                nc.tensor.matmul(
                                    c1[r],
                                    _ap(w1, k2 * 256, [[128, 2], [1, 128]]),
                                    rhs, start=(dy + dx == 0),
                                    stop=(dy == 2 and dx == 2), perf_mode=DRM)
                    for r in range(GR):
                        j = (ct * GR + r) * 6
                        nc.vector.bn_stats(stat1[:, j:j + 6], c1[r])
                st[g] = {'c1s': c1s, 'stat1': stat1}

            def s2_gn1_relu(g):
                d = st[g]
                A1, B1 = _gn_stats_ab(nc, cs, sps, d['stat1'], gm_mu, gm_cv1,
                                      gn1s, gn1b, AF, AL, f32, eps_t, GR, '1')
                pad2 = [cp.tile([128, 2 * 9 * 66], f8, tag=f"pad2_{r}",
                                name=f"pad2_{g}_{r}") for r in range(GR)]
                for r in range(GR):
                    nc.gpsimd.memset(_ap(pad2[r], 0, [[594, 2], [1, 66]]), 0.0)
                    nc.gpsimd.memset(
                        _ap(pad2[r], 0, [[594, 2], [66, 9], [65, 2]]), 0.0)
                for ct in range(2):
                    for r in range(GR):
                        nc.scalar.activation(
                            out=_ap(pad2[r], ct * 594 + 67, [[66, 8], [1, 64]]),
                            in_=d['c1s'][ct][r], func=AF.Relu,
                            bias=B1[:, ct * GR + r:ct * GR + r + 1],
                            scale=A1[:, ct * GR + r:ct * GR + r + 1])
                d['pad2'] = pad2

            def s3_conv2(g):
                d = st[g]
                pad2 = d['pad2']
                stat2 = cs.tile([128, 24], f32, tag="stat2", name=f"st2_{g}")
                c2s = {}
                for ct in range(2):
                    c2t = cps.tile([128, 256 * GR], f32, tag="c2",
                                   name=f"c2_{g}_{ct}")
                    c2 = [_ap(c2t, r * 256, [[1, 256]]) for r in range(GR)]
                    c2s[ct] = c2
                    for dy in range(3):
                        for dx in range(3):
                            k2 = (dy * 3 + dx) * 2 + ct
                            for r in range(GR):
                                rhs = _ap(pad2[r], dy * 66 + dx,
                                          [[594, 2], [132, 4], [1, 64]])
                                nc.tensor.matmul(
                                    c2[r],
                                    _ap(w2, k2 * 256, [[128, 2], [1, 128]]),
                                    rhs, start=(dy + dx == 0),
                                    stop=(dy == 2 and dx == 2), perf_mode=DRM)
                    for r in range(GR):
                        j = (ct * GR + r) * 6
                        nc.vector.bn_stats(stat2[:, j:j + 6], c2[r])
                d['stat2'] = stat2
                d['c2s'] = c2s

            def s4_gn2_seq(g):
                d = st[g]
                A2, B2 = _gn_stats_ab(nc, cs, sps, d['stat2'], gm_mu, gm_cv2,
                                      gn2s, gn2b, AF, AL, f32, eps_t, GR, '2')
                for ct in range(2):
                    for r in range(GR):
                        c2n = cs.tile([128, 256], bf16, tag=f"c2n{ct}",
                                      name=f"c2n{g}_{ct}")
                        nc.scalar.activation(
                            out=c2n, in_=d['c2s'][ct][r], func=AF.Relu,
                            bias=B2[:, ct * GR + r:ct * GR + r + 1],
                            scale=A2[:, ct * GR + r:ct * GR + r + 1])
                        hs = cs.tile([128, 128], f32, tag=f"hs{ct}",
                                     name=f"hs{g}_{ct}")
                        nc.gpsimd.tensor_add(hs, c2n[:, 0:128],
                                             c2n[:, 128:256])
                        nc.gpsimd.tensor_add(
                            seq[ct][:, (g * GR + r) * T:(g * GR + r + 1) * T],
                            hs[:, 0:64], hs[:, 64:128])
                del st[g]

            # emission order per iteration: s4(g-3), s2(g-1), s1(g), s3(g-2)
            # keeps the gs-psum consumers early in the DVE stream so PE's
            # group matmuls never wait on a stale gs buffer
            stages = ((3, s4_gn2_seq), (1, s2_gn1_relu),
                      (0, s1_load_conv1), (2, s3_conv2))
            for k in range(ngrp + 3):
                for si, fn in stages:
                    g = k - si
                    if 0 <= g < ngrp:
                        fn(g)
                if k == 0:
                    load_rest()
        if not _rest:
            load_rest()
        wih = _rest['wih']; whh = _rest['whh']; lstm_bt = _rest['lstm_bt']
        elT = _rest['elT']; el_b = _rest['el_b']; et1 = _rest['et1']
        wc2T = _rest['wc2T']; comb_b = _rest['comb_b']
        gwihT = _rest['gwihT']; gwhhT = _rest['gwhhT']
        if has_gru_bias:
            gru_bi = _rest['gru_bi']; gru_bh = _rest['gru_bh']
            ones_n = _rest['ones_n']
        owT = _rest['owT']; out_b = _rest['out_b']; vat8 = _rest['vat8']
        ones_f = _rest['ones_f']; ident_f = _rest['ident_f']
        ident_b = _rest['ident_b']
        tmask = _rest['tmask']; onehot0 = _rest['onehot0']

        if dbg:
            for ct in range(2):
                tmp = seq_p.tile([128, TN], f32, tag=f"dbgs{ct}", name=f"dbgs{ct}")
                nc.vector.tensor_copy(tmp, seq[ct])
                dma(out=dbg['seq'][ct, :, :], in_=tmp)

        # ================= LSTM phase =================
        if 'lstm' not in phases:
            for d in range(2):
                nc.vector.memset(hist[d], 0.01)
        if 'lstm' in phases:
         with tc.tile_pool(name="xpp", bufs=1) as xp, \
             tc.tile_pool(name="lst", bufs=1) as lsp:
            # xproj[d]: [128, 8*TN], blocks [i0 i1 f0 f1 o0 o1 g0 g1], col t*nr+n
            colmap = {0: 0, 1: 1, 2: 2, 3: 3, 4: 6, 5: 7, 6: 4, 7: 5}
            xproj = [xp.tile([128, 8 * TN], bf16, tag=f"xp{d}", name=f"xp{d}")
                     for d in range(2)]
            with tc.tile_pool(name="xps", bufs=1, space="PSUM") as xps:
                nch = TN // 512
                tch = 512 // nr
                for d in range(2):
                    for mt in range(8):
                        pss = [xps.tile([128, 512], f32, tag=f"xpps{ch}",
                                        name=f"xpps{ch}")
                               for ch in range(nch)]
                        for kt in range(2):
                            for ch in range(nch):
                                rhs = _ap(seq[kt], ch * tch,
                                          [[1, tch], [64, nr]])
                                nc.tensor.matmul(
                                    pss[ch],
                                    wih[d][:, (kt * 8 + mt) * 128:
                                           (kt * 8 + mt) * 128 + 128],
                                    rhs, start=(kt == 0), stop=(kt == 1))
                        for ch in range(nch):
                            o0 = colmap[mt] * TN + ch * 512
                            nc.scalar.activation(
                                out=xproj[d][:, o0:o0 + 512], in_=pss[ch],
                                func=AF.Identity,
                                bias=lstm_bt[d][:, mt:mt + 1])

            # h lives directly in hist; hzero holds the step-0 state
            hzero = lsp.tile([128, 2 * nr], bf16, tag="hzero", name="hzero")
            cst = [lsp.tile([128, 2 * nr], f32, tag=f"cst{d}", name=f"cst{d}")
                   for d in range(2)]
            nc.vector.memset(hzero, 0.0)
            for d in range(2):
                nc.vector.memset(cst[d], 0.0)

            with tc.tile_pool(name="gps", bufs=2, space="PSUM") as gpsp:
                for step in range(T):
                    for d in range(2):
                        t = step if d == 0 else T - 1 - step
                        gps = gpsp.tile([128, 8 * nr], f32, tag=f"g{d}", name=f"g{d}")
                        tprev = (t - 1) if d == 0 else (t + 1)
                        for mt in range(8):
                            cb = colmap[mt] * nr
                            for kt in range(2):
                                if step == 0:
                                    hrhs = hzero[:, kt * nr:(kt + 1) * nr]
                                else:
                                    hrhs = _ap(hist[d], kt * TN + tprev * nr,
                                               [[1, nr]])
                                nc.tensor.matmul(
                                    gps[:, cb:cb + nr],
                                    whh[d][:, (kt * 8 + mt) * 128:
                                           (kt * 8 + mt) * 128 + 128],
                                    hrhs,
                                    start=(kt == 0), stop=False,
                                    skip_group_check=True)
                        # fold x-projection in on the PE (frees the DVE add)
                        nc.tensor.matmul(
                            gps, ident_b,
                            _ap(xproj[d], t * nr, [[TN, 8], [1, nr]]),
                            start=False, stop=True, skip_group_check=True)
                        sgi = lsp.tile([128, 6 * nr], f32, tag=f"sgi{d}",
                                       name=f"sgi{d}")
                        tgg = lsp.tile([128, 2 * nr], f32, tag=f"tgg{d}",
                                       name=f"tgg{d}")
                        # i/f gates first so the c-state chain starts sooner
                        nc.scalar.activation(sgi[:, 0:4 * nr],
                                             gps[:, 0:4 * nr], AF.Sigmoid)
                        nc.scalar.activation(tgg, gps[:, 6 * nr:8 * nr],
                                             AF.Tanh)
                        nc.scalar.activation(sgi[:, 4 * nr:6 * nr],
                                             gps[:, 4 * nr:6 * nr],
                                             AF.Sigmoid)
                        tmp = lsp.tile([128, 2 * nr], f32, tag=f"tmp{d}", name=f"tmp{d}")
                        nc.vector.tensor_mul(tmp, sgi[:, 2 * nr:4 * nr], cst[d])
                        nc.vector.tensor_mul(tgg, sgi[:, 0:2 * nr], tgg)
                        nc.vector.tensor_add(cst[d], tmp, tgg)
                        tct = lsp.tile([128, 2 * nr], f32, tag=f"tct{d}", name=f"tct{d}")
                        nc.scalar.activation(tct, cst[d], AF.Tanh)
                        nc.vector.tensor_mul(
                            _ap(hist[d], t * nr, [[TN, 2], [1, nr]]),
                            sgi[:, 4 * nr:6 * nr], tct)

        if dbg:
            for d in range(2):
                tmp = hist_p.tile([128, 2 * TN], f32, tag=f"dbgh{d}", name=f"dbgh{d}")
                nc.vector.tensor_copy(tmp, hist[d])
                dma(out=dbg['hist'][d, :, :], in_=tmp)

        # ================= enc =================
        # enc_nt: [128, 2TN] ct-major, n-major inside (col ct*TN + n*T + t)
        # enc_tn: [128, 2TN] ct-major, t-major inside (col ct*TN + t*nr + n)
        enc_p = ctx.enter_context(tc.tile_pool(name="encp", bufs=1))
        enc_nt = enc_p.tile([128, 2 * TN], bf16, tag="ent", name="ent")
        enc_tn = enc_p.tile([128, 2 * TN], bf16, tag="etn", name="etn")
        with tc.tile_pool(name="eps", bufs=1, space="PSUM") as eps:
            NCH = TN // 512
            for ct in range(2):
                pss = [eps.tile([128, 512], f32, tag=f"encps{ch}",
                                name=f"encps{ch}")
                       for ch in range(NCH)]
                for kq in range(4):
                    d, kt = divmod(kq, 2)
                    for ch in range(NCH):
                        rhs = _ap(hist[d], kt * TN + ch * 8,
                                  [[1, 8], [nr, T]])
                        nc.tensor.matmul(
                            pss[ch], elT[:, (kq * 2 + ct) * 128:
                                         (kq * 2 + ct) * 128 + 128],
                            rhs, start=(kq == 0), stop=(kq == 3))
                for ch in range(NCH):
                    nc.scalar.activation(
                        out=enc_nt[:, ct * TN + ch * 512:
                                   ct * TN + ch * 512 + 512],
                        in_=pss[ch], func=AF.Identity,
                        bias=el_b[:, ct:ct + 1])
                    nc.scalar.activation(
                        out=_ap(enc_tn, ct * TN + ch * 8, [[1, 8], [nr, T]]),
                        in_=pss[ch], func=AF.Identity,
                        bias=el_b[:, ct:ct + 1])

        # ================= decoder =================
        # 2 independent roi streams; per step: fp8-DR scores with replicated
        # rows (no aw broadcast), softmax denominator folded after the
        # t-reduction, direct Sigmoid, partition_all_reduce argmax, deferred
        # log-sum-exp.
        from concourse import bass_isa
        DRM2 = mybir.MatmulPerfMode.DoubleRow
        f8 = dt.float8e4
        NS = 2
        NRS = nr // NS
        TNS = NRS * T
        dp = ctx.enter_context(tc.tile_pool(name="decp", bufs=2))
        accp = ctx.enter_context(tc.tile_pool(name="accp", bufs=1))
        sebuf = accp.tile([1, NUMCH * nr], f32, tag="sebuf", name="sebuf")
        acc_tgt = [accp.tile([VOC, 1], f32, tag=f"atg{s}", name=f"atg{s}")
                   for s in range(NS)]
        hid_bf = [accp.tile([128, 2 * NRS], bf16, tag=f"hb{s}", name=f"hb{s}")
                  for s in range(NS)]
        onehot = [accp.tile([VOC, NRS], bf16, tag=f"oh{s}", name=f"oh{s}")
                  for s in range(NS)]
        # logits live on all 128 partitions (pad rows at -1e30) so the
        # argmax partition_all_reduce can use power-of-two channels
        lsbt = [accp.tile([128, NRS], f32, tag=f"lsb{s}", name=f"lsb{s}")
                for s in range(NS)]
        for s in range(NS):
            nc.vector.memset(acc_tgt[s], 0.0)
            nc.vector.memset(hid_bf[s], 0.0)
            nc.vector.memset(lsbt[s], -1e30)
            nc.vector.tensor_copy(onehot[s],
                                  onehot0[:, s * NRS:(s + 1) * NRS])

        NUMCH_eff = NUMCH if 'dec' in phases else 0
        with tc.tile_pool(name="dpsA", bufs=1, space="PSUM") as dpsA, \
             tc.tile_pool(name="dpsC", bufs=2, space="PSUM") as dpsC, \
             tc.tile_pool(name="dpsG", bufs=1, space="PSUM") as dpsG:
            big = [dpsA.tile([128, TNS], f32, tag=f"big{s}", name=f"big{s}")
                   for s in range(NS)]
            _es = {}

            def dec_ph1(s, step):
              # chain-critical: schedule the attention front-end ASAP
              with tc.high_priority(offset=2000):
                # A = tanh(enc + hid), t-major, fp8 out for DR scores
                Aad = dp.tile([128, 2 * TNS], bf16, tag=f"Aad{s}",
                              name=f"Aad{s}")
                At = dp.tile([128, 2 * TNS], f8, tag=f"At{s}", name=f"At{s}")
                e = dp.tile([128, TNS], bf16, tag=f"e{s}", name=f"e{s}")
                # per 8-roi chunk j: add -> tanh -> scores -> exp flow
                # independently, so chunk j=1 overlaps chunk j=0's tail
                for j in range(NRS // 8):
                    for ct in range(2):
                        av = _ap(Aad, ct * TNS + j * 8, [[NRS, T], [1, 8]])
                        nc.vector.tensor_add(
                            av,
                            _ap(enc_tn, ct * TN + s * NRS + j * 8,
                                [[nr, T], [1, 8]]),
                            _ap(hid_bf[s], ct * NRS + j * 8,
                                [[0, T], [1, 8]]))
                        nc.scalar.activation(
                            _ap(At, ct * TNS + j * 8, [[NRS, T], [1, 8]]),
                            av, AF.Tanh)
                    rhs = _ap(At, j * 8, [[TNS, 2], [1, 8], [NRS, T]])
                    nc.tensor.matmul(big[s][:, j * 512:(j + 1) * 512],
                                     _ap(vat8, 0, [[128, 2], [1, 128]]),
                                     rhs, start=True, stop=True,
                                     perf_mode=DRM2)
                    nc.scalar.activation(e[:, j * 512:(j + 1) * 512],
                                         big[s][:, j * 512:(j + 1) * 512],
                                         AF.Exp, scale=1.0 / 16.0)
                # ctx numerator chunk follows its exp chunk immediately
                P = dp.tile([128, 2 * TNS], bf16, tag=f"P{s}", name=f"P{s}")
                for j in range(2):
                    nc.vector.tensor_mul(
                        _ap(P, j * 512, [[TNS, 2], [1, 512]]),
                        _ap(enc_nt, s * TNS + j * 512, [[TN, 2], [1, 512]]),
                        _ap(e, j * 512, [[0, 2], [1, 512]]))
                _es[s] = (e, P)

            def dec_ph2(s, step):
                e, P = _es[s]
                P2 = dp.tile([128, TNS], bf16, tag=f"P2{s}", name=f"P2{s}")
                P4 = dp.tile([128, TNS // 2], bf16, tag=f"P4{s}",
                             name=f"P4{s}")
                ctxr = dp.tile([128, 2 * NRS], f32, tag=f"cxr{s}",
                               name=f"cxr{s}")
                for j in range(2):
                    nc.vector.tensor_add(
                        _ap(P2, j * 256, [[TNS // 2, 2], [1, 256]]),
                        _ap(P, j * 512, [[TNS, 2], [T, 8], [1, 32]]),
                        _ap(P, j * 512 + 32, [[TNS, 2], [T, 8], [1, 32]]))
                    nc.vector.tensor_add(
                        _ap(P4, j * 128, [[TNS // 4, 2], [1, 128]]),
                        _ap(P2, j * 256, [[TNS // 2, 2], [32, 8], [1, 16]]),
                        _ap(P2, j * 256 + 16,
                            [[TNS // 2, 2], [32, 8], [1, 16]]))
                    nc.vector.tensor_reduce(
                        _ap(ctxr, j * 8, [[NRS, 2], [1, 8]]),
                        _ap(P4, j * 128, [[TNS // 4, 2], [16, 8], [1, 16]]),
                        axis=AX.X, op=AL.add)
                e2 = dp.tile([128, TNS // 2], bf16, tag=f"e2{s}",
                             name=f"e2{s}")
                nc.vector.tensor_add(e2, _ap(e, 0, [[T, NRS], [1, 32]]),
                                     _ap(e, 32, [[T, NRS], [1, 32]]))
                esum = dp.tile([128, NRS], f32, tag=f"es{s}", name=f"es{s}")
                nc.vector.tensor_reduce(
                    esum, _ap(e2, 0, [[32, NRS], [1, 32]]), axis=AX.X,
                    op=AL.add)
                rec = dp.tile([128, NRS], f32, tag=f"rc{s}", name=f"rc{s}")
                nc.vector.reciprocal(rec, esum)
                ctx_bf = dp.tile([128, 2 * NRS], bf16, tag=f"cxb{s}",
                                 name=f"cxb{s}")
                nc.vector.tensor_mul(ctx_bf, ctxr,
                                     _ap(rec, 0, [[0, 2], [1, NRS]]))
                # combine
                comb_bf = dp.tile([128, 2 * NRS], bf16, tag=f"cb{s}",
                                  name=f"cb{s}")
                for mt in range(2):
                    cpsd = dpsC.tile([128, NRS], f32, tag="small",
                                     name=f"cps{s}")
                    nc.tensor.matmul(cpsd, et1[:, mt * 128:mt * 128 + 128],
                                     onehot[s], start=True, stop=False)
                    for kt in range(2):
                        nc.tensor.matmul(
                            cpsd,
                            wc2T[:, (kt * 2 + mt) * 128:
                                 (kt * 2 + mt) * 128 + 128],
                            ctx_bf[:, kt * NRS:(kt + 1) * NRS],
                            start=False, stop=(kt == 1))
                    nc.scalar.activation(
                        out=comb_bf[:, mt * NRS:(mt + 1) * NRS],
                        in_=cpsd, func=AF.Relu, bias=comb_b[:, mt:mt + 1])
                # GRU: r,z input+hidden projections accumulate jointly
                gall = dpsG.tile([128, 8 * NRS], f32, tag=f"gal{s}",
                                 name=f"gal{s}")
                grz = gall[:, 0:4 * NRS]
                gin = gall[:, 4 * NRS:6 * NRS]
                ghn = gall[:, 6 * NRS:8 * NRS]
                nb = not has_gru_bias
                for mt in range(4):
                    oreg = grz[:, mt * NRS:(mt + 1) * NRS]
                    # hidden projections first: hid is ready long before comb
                    for kt in range(2):
                        nc.tensor.matmul(
                            oreg,
                            gwhhT[:, (kt * 6 + mt) * 128:
                                  (kt * 6 + mt) * 128 + 128],
                            hid_bf[s][:, kt * NRS:(kt + 1) * NRS],
                            start=(kt == 0), stop=False)
                    for kt in range(2):
                        nc.tensor.matmul(
                            oreg,
                            gwihT[:, (kt * 6 + mt) * 128:
                                  (kt * 6 + mt) * 128 + 128],
                            comb_bf[:, kt * NRS:(kt + 1) * NRS],
                            start=False, stop=(kt == 1 and nb))
                    if has_gru_bias:
                        nc.tensor.matmul(oreg,
                                         gru_bi[:, mt * 128:mt * 128 + 128],
                                         ones_n[:, 0:NRS],
                                         start=False, stop=False)
                        nc.tensor.matmul(oreg,
                                         gru_bh[:, mt * 128:mt * 128 + 128],
                                         ones_n[:, 0:NRS],
                                         start=False, stop=True)
                for mt in range(4, 6):
                    j = (mt - 4) * NRS
                    for kt in range(2):
                        nc.tensor.matmul(
                            gin[:, j:j + NRS],
                            gwihT[:, (kt * 6 + mt) * 128:
                                  (kt * 6 + mt) * 128 + 128],
                            comb_bf[:, kt * NRS:(kt + 1) * NRS],
                            start=(kt == 0), stop=(kt == 1 and nb))
                        nc.tensor.matmul(
                            ghn[:, j:j + NRS],
                            gwhhT[:, (kt * 6 + mt) * 128:
                                  (kt * 6 + mt) * 128 + 128],
                            hid_bf[s][:, kt * NRS:(kt + 1) * NRS],
                            start=(kt == 0), stop=(kt == 1 and nb))
                    if has_gru_bias:
                        nc.tensor.matmul(gin[:, j:j + NRS],
                                         gru_bi[:, mt * 128:mt * 128 + 128],
                                         ones_n[:, 0:NRS],
                                         start=False, stop=True)
                        nc.tensor.matmul(ghn[:, j:j + NRS],
                                         gru_bh[:, mt * 128:mt * 128 + 128],
                                         ones_n[:, 0:NRS],
                                         start=False, stop=True)
                with tc.high_priority(offset=1000):
                    rz = dp.tile([128, 4 * NRS], f32, tag=f"rz{s}",
                                 name=f"rz{s}")
                    nc.scalar.activation(rz, grz, AF.Sigmoid)
                    t1 = dp.tile([128, 2 * NRS], f32, tag=f"t1{s}",
                                 name=f"t1{s}")
                    nc.vector.tensor_mul(t1, rz[:, 0:2 * NRS], ghn)
                    nnt = dp.tile([128, 2 * NRS], f32, tag=f"nt{s}",
                                  name=f"nt{s}")
                    nc.vector.scalar_tensor_tensor(
                        out=nnt, in0=gin, scalar=0.0, in1=t1,
                        op0=AL.bypass, op1=AL.add)
                    nc.scalar.activation(nnt, nnt, AF.Tanh)
                    dd = dp.tile([128, 2 * NRS], f32, tag=f"dd{s}",
                                 name=f"dd{s}")
                    nc.vector.tensor_sub(dd, hid_bf[s], nnt)
                    nc.vector.tensor_mul(dd, rz[:, 2 * NRS:4 * NRS], dd)
                    nc.vector.tensor_add(hid_bf[s], nnt, dd)

            def dec_ph3(s, step):
                # logits + loss + argmax-onehot (off the critical path)
                lg = dpsC.tile([VOC, NRS], f32, tag="small", name=f"lg{s}")
                for kt in range(2):
                    nc.tensor.matmul(lg, owT[:, kt * VOC:(kt + 1) * VOC],
                                     hid_bf[s][:, kt * NRS:(kt + 1) * NRS],
                                     start=(kt == 0), stop=(kt == 1))
                lsb = lsbt[s][0:VOC, :]
                nc.scalar.activation(lsb, lg, AF.Identity, bias=out_b[:, 0:1])
                if step < NUMCH - 1:
                    lgT_ps = dpsC.tile([NRS, VOC], f32, tag="small",
                                       name=f"lgT{s}")
                    nc.tensor.transpose(lgT_ps, lsb, ident_f[0:VOC, 0:VOC])
                    lgT = dp.tile([NRS, VOC], f32, tag=f"lgT{s}",
                                  name=f"lgTs{s}")
                    nc.vector.tensor_copy(lgT, lgT_ps)
                    mx8 = dp.tile([NRS, 8], f32, tag=f"mx{s}", name=f"mx{s}")
                    nc.vector.max(out=mx8, in_=lgT)
                    mT = dp.tile([NRS, VOC], f32, tag=f"mT{s}", name=f"mT{s}")
                    nc.vector.tensor_scalar(out=mT, in0=lgT,
                                            scalar1=mx8[:, 0:1], scalar2=None,
                                            op0=AL.is_equal)
                    oh_ps = dpsC.tile([VOC, NRS], f32, tag="small",
                                      name=f"ohp{s}")
                    nc.tensor.transpose(oh_ps, mT, ident_f[0:NRS, 0:NRS])
                    nc.vector.tensor_copy(onehot[s], oh_ps)
                ex = dp.tile([VOC, NRS], f32, tag=f"ex{s}", name=f"ex{s}")
                nc.scalar.activation(ex, lsb, AF.Exp)
                se_ps = dpsC.tile([1, NRS], f32, tag="small", name=f"se{s}")
                nc.tensor.matmul(se_ps, ones_f[0:VOC, 0:1], ex,
                                 start=True, stop=True)
                nc.vector.tensor_copy(
                    sebuf[:, step * nr + s * NRS:step * nr + s * NRS + NRS],
                    se_ps)
                junk = dp.tile([VOC, NRS], f32, tag=f"jk{s}", name=f"jk{s}")
                ttmp = dp.tile([VOC, 1], f32, tag=f"tt{s}", name=f"tt{s}")
                nc.vector.scalar_tensor_tensor(
                    out=junk, in0=lsb, scalar=0.0,
                    in1=tmask[:, step * nr + s * NRS:
                              step * nr + s * NRS + NRS],
                    op0=AL.bypass, op1=AL.mult, accum_out=ttmp)
                nc.gpsimd.tensor_add(acc_tgt[s], acc_tgt[s], ttmp)

            # stream-interleaved emission: while stream s waits on its
            # attention chain (tanh->scores->exp), the other stream's
            # vector work keeps DVE's in-order queue busy; the logits/loss
            # block (ph3) is deferred to the iteration tail so it never
            # delays the next tanh/exp in the Act queue
            for step in range(NUMCH_eff):
                dec_ph1(0, step)
                if step > 0:
                    dec_ph2(1, step - 1)
                dec_ph1(1, step)
                dec_ph2(0, step)
                if step > 0:
                    dec_ph3(1, step - 1)
                dec_ph3(0, step)
            if NUMCH_eff:
                dec_ph2(1, NUMCH_eff - 1)
                dec_ph3(1, NUMCH_eff - 1)
        if 'dec' not in phases:
            nc.vector.memset(sebuf, 1.0)
        lse_ln = accp.tile([1, NUMCH * nr], f32, tag="lse_ln", name="lse_ln")
        nc.scalar.activation(lse_ln, sebuf, AF.Ln)
        acc_lse = accp.tile([1, nr], f32, tag="acc_lse", name="acc_lse")
        nc.vector.tensor_reduce(
            acc_lse, _ap(lse_ln, 0, [[1, nr], [nr, NUMCH]]),
            axis=AX.X, op=AL.add)
        nc.vector.tensor_add(acc_tgt[0], acc_tgt[0], acc_tgt[1])
        dma(out=out_parts[0:nr, :], in_=acc_lse)
        dma(out=out_parts[nr:nr + VOC, :], in_=acc_tgt[0])


def _gn_stats_ab(nc, pool, psum_pool, stat, gm_mu, gm_cv, gn_s, gn_b,
                 AF, AL, f32, eps_t, R, tagsfx):
    """GroupNorm A/B from bn_stats outputs for both cts of one conv layer.

    stat: [128, 12R] = per (ct,r) the 6 bn_stats cols
    (cnt_e, mean_e, cnt_e*var_e, cnt_o, mean_o, cnt_o*var_o).
    Returns A, B [128, 2R] ((ct,r)-major): A = s*rstd, B = b - mu*A.
    """
    M = 2 * R
    msq = pool.tile([128, 2 * M], f32, tag="msq" + tagsfx, name="msq" + tagsfx)
    mv = _ap(stat, 1, [[6, M], [3, 2]])
    nc.vector.tensor_mul(msq, mv, mv)
    gs = psum_pool.tile([128, 2 * M], f32, tag="gs", name="gs" + tagsfx)
    mu_ap = _ap(gs, 0, [[1, M]])
    e2_ap = _ap(gs, M, [[1, M]])
    # mu = sum_p (mean_e + mean_o)/16
    nc.tensor.matmul(mu_ap, gm_mu, _ap(stat, 1, [[6, M]]), start=True,
                     stop=False)
    nc.tensor.matmul(mu_ap, gm_mu, _ap(stat, 4, [[6, M]]), start=False,
                     stop=True)
    # E2 = sum_p [(cv_e+cv_o)/(8 cnt) + (msq_e+msq_o)/16]
    nc.tensor.matmul(e2_ap, gm_cv, _ap(stat, 2, [[6, M]]), start=True,
                     stop=False)
    nc.tensor.matmul(e2_ap, gm_cv, _ap(stat, 5, [[6, M]]), start=False,
                     stop=False)
    nc.tensor.matmul(e2_ap, gm_mu, _ap(msq, 0, [[2, M]]), start=False,
                     stop=False)
    nc.tensor.matmul(e2_ap, gm_mu, _ap(msq, 1, [[2, M]]), start=False,
                     stop=True)
    A = pool.tile([128, M], f32, tag="gnA" + tagsfx, name="gnA" + tagsfx)
    B = pool.tile([128, M], f32, tag="gnB" + tagsfx, name="gnB" + tagsfx)
    muE = pool.tile([128, 2 * M], f32, tag="gnm" + tagsfx, name="gnm" + tagsfx)
    var = pool.tile([128, M], f32, tag="gnv" + tagsfx, name="gnv" + tagsfx)
    nc.vector.tensor_copy(muE, gs)
    mu = muE[:, 0:M]
    e2 = muE[:, M:2 * M]
    nc.vector.tensor_mul(var, mu, mu)
    nc.vector.scalar_tensor_tensor(out=var, in0=e2, scalar=0.0, in1=var,
                                   op0=AL.bypass, op1=AL.subtract)
    nc.scalar.activation(var, var, AF.Sqrt, bias=eps_t[:, 0:1])
    nc.vector.reciprocal(var, var)
    # A = rstd * s  (s broadcast per ct across the R rois)
    nc.vector.tensor_mul(A, var, _ap(gn_s, 0, [[1, 2], [0, R]]))
    nc.vector.tensor_mul(var, mu, A)
    # B = b - mu*A
    nc.vector.scalar_tensor_tensor(
        out=B, in0=_ap(gn_b, 0, [[1, 2], [0, R]]), scalar=0.0, in1=var,
        op0=AL.bypass, op1=AL.subtract)
    return A, B


def _gn_ab4(nc, pool, gs, s_col, b_col, AF, AL, f32, eps_t, R):
    """gs: psum [128, 2R] = [mu_r0, E2_r0, ...] for one ct across R rois.
    Returns (A, B) tiles [128, R]: A = rstd*s, B = b - mu*A."""
    A = pool.tile([128, R], f32, tag="gnA", name="gnA")
    B = pool.tile([128, R], f32, tag="gnB", name="gnB")
    mu = pool.tile([128, R], f32, tag="gnmu", name="gnmu")
    var = pool.tile([128, R], f32, tag="gnvar", name="gnvar")
    nc.vector.tensor_copy(mu, _ap(gs, 0, [[2, R]]))
    nc.vector.tensor_mul(var, mu, mu)
    nc.vector.scalar_tensor_tensor(
        out=var, in0=_ap(gs, 1, [[2, R]]), scalar=0.0, in1=var,
        op0=AL.bypass, op1=AL.subtract)
    nc.scalar.activation(var, var, AF.Sqrt, bias=eps_t[:, 0:1])
    nc.vector.reciprocal(var, var)
    nc.vector.tensor_scalar_mul(A, var, s_col)
    nc.vector.tensor_mul(mu, mu, A)
    nc.vector.tensor_scalar(out=B, in0=mu, scalar1=b_col, scalar2=-1.0,
                            op0=AL.subtract, op1=AL.mult)
    return A, B


def _gn_ab(nc, pool, gs, s_const, b_const, AF, AL, f32, eps_t):
    """gs psum [128,4] = [mu0, E2_0, mu1, E2_1] -> AB [128,4] = [A0,A1,B0,B1]:
    A = rstd*s, B = b - mu*A."""
    AB = pool.tile([128, 4], f32, tag="AB", name="AB")
    mu = pool.tile([128, 2], f32, tag="gnmu", name="gnmu")
    var = pool.tile([128, 2], f32, tag="gnvar", name="gnvar")
    nc.vector.tensor_copy(mu, _ap(gs, 0, [[2, 2]]))
    nc.vector.tensor_mul(var, mu, mu)
    nc.vector.scalar_tensor_tensor(
        out=var, in0=_ap(gs, 1, [[2, 2]]), scalar=0.0, in1=var,
        op0=AL.bypass, op1=AL.subtract)
    nc.scalar.activation(var, var, AF.Sqrt, bias=eps_t[:, 0:1])
    nc.vector.reciprocal(var, var)
    nc.vector.tensor_mul(AB[:, 0:2], var, s_const)
    nc.vector.tensor_mul(mu, mu, AB[:, 0:2])
    nc.vector.scalar_tensor_tensor(
        out=AB[:, 2:4], in0=b_const, scalar=0.0, in1=mu,
        op0=AL.bypass, op1=AL.subtract)
    return AB


# ----------------------------------------------------------------------------
# driver
# ----------------------------------------------------------------------------

def kernel(**inputs):
    w = _prep_weights(inputs)
    maps = _per_core_maps(inputs, w)
    key = ('v2', w['has_gru_bias'])
    if key not in _PROG_CACHE:
        nc = build_program(NR, w['has_gru_bias'], debug=False)
        dedupe_ldweights(nc)
        split_excess_waits(nc)
        _PROG_CACHE[key] = nc
    nc = _PROG_CACHE[key]
    from concourse import bass_utils
    r = bass_utils.run_bass_kernel_spmd(nc, maps, core_ids=list(range(NCORES)))
    tot = 0.0
    for core in range(NCORES):
        parts = r.results[core]['out_parts'][:, 0].astype(np.float64)
        tot += parts[:NR].sum() - parts[NR:].sum()
    return np.float32(tot / N_FULL)




# revision 102
# speedup vs baseline: 1.0178x; 1.0178x over previous
"""Trainium2 Bass kernel for nn_ATTPredictor.

CRNN encoder (2x strided conv + GroupNorm + ReLU), BiLSTM (T=64), linear
projection, then a 25-step attention GRU decoder with greedy argmax feedback,
producing a scalar NLL loss.  Data-parallel over the 256 ROIs on 8 NeuronCores
(32 each, all weights replicated); per-core partial sums are combined on host.

Self-contained: only needs numpy / ml_dtypes / concourse (installed env).
"""
import numpy as np
import ml_dtypes

BF16 = ml_dtypes.bfloat16
FP8 = ml_dtypes.float8_e4m3
CONV_WSCALE = 16.0  # GN is scale-invariant; lifts fp8 weights out of subnormals

C = 256
VOC = 97
NUMCH = 25
N_FULL = 256
HH, WW = 16, 64
T = 64
NCORES = 8
NR = N_FULL // NCORES

_PROG_CACHE = {}


# ----------------------------------------------------------------------------
# walrus workarounds (this build rejects >1 sync wait per instruction)
# ----------------------------------------------------------------------------

def _patch_ldw_opt():
    # enable walrus fast-weight-load lowering (default-off in this harness);
    # numerically neutral, speeds up the Ldweights-bound conv/LSTM phases
    from concourse import bass_utils as bu
    if getattr(bu, '_attp_ldw_patched', False):
        return
    orig = bu.run_command

    def patched(argv, **kw):
        argv = ["--enable-ldw-opt=true" if a == "--enable-ldw-opt=false" else a
                for a in argv]
        return orig(argv, **kw)

    bu.run_command = patched
    bu._attp_ldw_patched = True


def _patch_tile_drain():
    import concourse.tile as tile_mod
    from concourse.vector_clock import ScopedClock, VectorClock

    if getattr(tile_mod.TileContext, '_attp_patched', False):
        return

    def _split_drain_and_barrier(self, tick_clock, wait_clock):
        gc = tick_clock.global_clock
        n = len(gc)
        for i in range(n):
            vals = [gc[p] if p == i else 0 for p in range(n)]
            if not any(vals):
                continue
            d = self.nc.sync.drain()
            wait_clock.add_sem_waits(d.ins, ScopedClock({None: VectorClock(vals)}))
        self.nc.all_engine_barrier()
        popped = self.nc._tile_sem_poison_stack.pop()
        assert popped is self._sem_poison
        self.nc.clear_and_free_semaphores(list(self.sems.allocated().values()))
        self.nc.all_engine_barrier()

    tile_mod.TileContext._drain_and_barrier = _split_drain_and_barrier
    tile_mod.TileContext._attp_patched = True


def split_excess_waits(nc):
    """Hoist extra sync waits onto sem-only EventSemaphore instructions."""
    import bass_rust
    from concourse import mybir

    k = 0
    for bbname, bb in nc.bb_map.items():
        insts = bb.bb.instructions
        idx = 0
        while idx < len(insts):
            i = insts[idx]
            si = i.sync_info
            if si is not None and len(si.on_wait) > 1:
                waits = list(si.on_wait)
                for w in waits[:-1]:
                    es = mybir.InstEventSemaphore(
                        name=f"wsplit_{k}", ins=[], outs=[])
                    k += 1
                    es.engine = i.engine
                    es.sync_info = bass_rust.SyncInfo(on_wait=[w], on_update=[])
                    insts.insert(idx, es)
                    idx += 1
                si.on_wait = waits[-1:]
                i.sync_info = si
            idx += 1
    return k


def dedupe_ldweights(nc):
    """Remove Ldweights whose weight AP/position matches the immediately
    preceding PE weight-load state (weights stay resident in the array)."""
    removed = 0
    for bbname, bb in nc.bb_map.items():
        insts = bb.bb.instructions
        last_sig = None
        keep = []
        for i in insts:
            eng = str(i.engine)
            if not eng.endswith('PE'):
                keep.append(i)
                continue
            op = i.concise_opcode()
            if op == 'Ldweights':
                try:
                    sig = (str(i.ins[0]), str(getattr(i, 'tile_position', None)),
                           str(getattr(i, 'tile_size', None)))
                except Exception:
                    sig = None
                si = i.sync_info
                has_sync = si is not None and (len(si.on_wait) > 0
                                               or len(si.on_update) > 0)
                if sig is not None and sig == last_sig and not has_sync:
                    removed += 1
                    continue
                last_sig = sig
                keep.append(i)
            elif op == 'Matmult':
                if getattr(i, 'is_transpose', False):
                    last_sig = None
                keep.append(i)
            else:
                keep.append(i)
        if removed:
            insts[:] = keep
    return removed


# ----------------------------------------------------------------------------
# host-side preparation
# ----------------------------------------------------------------------------

def _prep_weights(inp):
    f32 = np.float32
    w = {}

    rois = np.asarray(inp['rois'], f32)
    rp = np.zeros((2, 128, N_FULL, 17, 66), FP8)
    rc = rois.transpose(1, 0, 2, 3).reshape(2, 128, N_FULL, HH, WW)
    rp[:, :, :, 1:17, 1:65] = rc.astype(FP8)
    w['rois_cm'] = np.ascontiguousarray(rp.transpose(1, 0, 2, 3, 4))

    def conv_lhsT8(cw):
        # [i, (dy,dx,ct), kt, o] fp8 DoubleRow layout (kt pairs fold into K)
        a = (np.asarray(cw, f32) * CONV_WSCALE).reshape(2, 128, 2, 128, 3, 3)
        b = a.transpose(3, 4, 5, 0, 2, 1)        # i, dy, dx, ct, kt, o
        return np.ascontiguousarray(b.astype(FP8)).reshape(128, -1)

    w['w1T'] = conv_lhsT8(inp['conv1_w'])
    w['w2T'] = conv_lhsT8(inp['conv2_w'])

    def gmat(scale):
        gm = np.zeros((128, 128), f32)
        for i in range(128):
            gm[i, (i // 8) * 8:(i // 8) * 8 + 8] = scale
        return gm
    # bn_stats-based stats: mu = sum_p (mean_e+mean_o)/16;
    # E2 = sum_p [(cvar_e+cvar_o)/(8*cnt) + (msq_e+msq_o)/16]
    w['gmat_mu'] = gmat(1.0 / 16.0)
    w['gmat_cv1'] = gmat(1.0 / (8 * 512.0))
    w['gmat_cv2'] = gmat(1.0 / (8 * 256.0))
    for nm in ('gn1_s', 'gn1_b', 'gn2_s', 'gn2_b'):
        w[nm] = np.ascontiguousarray(
            np.asarray(inp[nm], f32).reshape(2, 128).T)

    def lin_lhsT(mat, scale=1.0):
        M, K = mat.shape
        nk, nm = K // 128, M // 128
        a = (np.asarray(mat, f32) * scale).astype(BF16)
        a = a.reshape(nm, 128, nk, 128)                      # mt,o,kt,i
        return np.ascontiguousarray(a.transpose(3, 2, 0, 1)).reshape(128, -1)

    w['wih_f'] = lin_lhsT(np.asarray(inp['lstm_wih_f']), 0.25)
    w['wih_b'] = lin_lhsT(np.asarray(inp['lstm_wih_b']), 0.25)
    w['whh_f'] = lin_lhsT(np.asarray(inp['lstm_whh_f']))
    w['whh_b'] = lin_lhsT(np.asarray(inp['lstm_whh_b']))
    w['lstm_bt_f'] = np.ascontiguousarray(
        (np.asarray(inp['lstm_bih_f'], f32)
         + np.asarray(inp['lstm_bhh_f'], f32)).reshape(8, 128).T)
    w['lstm_bt_b'] = np.ascontiguousarray(
        (np.asarray(inp['lstm_bih_b'], f32)
         + np.asarray(inp['lstm_bhh_b'], f32)).reshape(8, 128).T)

    w['elT'] = lin_lhsT(np.asarray(inp['emb_lin_w']))
    w['el_b'] = np.ascontiguousarray(
        np.asarray(inp['emb_lin_b'], f32).reshape(2, 128).T)

    cw = np.asarray(inp['combine_w'], f32)
    w['et1'] = (np.asarray(inp['embed_tab'], f32) @ cw[:, :C].T).astype(BF16)
    w['wc2T'] = lin_lhsT(cw[:, C:])
    w['comb_b'] = np.ascontiguousarray(
        np.asarray(inp['combine_b'], f32).reshape(2, 128).T)

    w['gwihT'] = lin_lhsT(np.asarray(inp['gru_wih']))
    w['gwhhT'] = lin_lhsT(np.asarray(inp['gru_whh']))
    w['gru_bi'] = np.asarray(inp['gru_bih'], f32).reshape(1, 768).astype(BF16)
    w['gru_bh'] = np.asarray(inp['gru_bhh'], f32).reshape(1, 768).astype(BF16)
    w['has_gru_bias'] = bool(np.any(np.asarray(inp['gru_bih']))
                             or np.any(np.asarray(inp['gru_bhh'])))

    ow = np.asarray(inp['out_w'], f32)                       # (97, 256)
    w['owT'] = np.ascontiguousarray(
        ow.T.reshape(2, 128, VOC).transpose(1, 0, 2).astype(BF16)
    ).reshape(128, 2 * VOC)
    w['out_b'] = np.asarray(inp['out_b'], f32).reshape(VOC, 1).copy()
    # vat8: [i, kt, m] fp8, x16, replicated along all 128 output columns so
    # the score matmul writes identical rows on every psum partition
    vat2 = np.asarray(inp['vat_w'], f32).reshape(2, 128).T * 16.0
    w['vat8'] = np.ascontiguousarray(
        np.broadcast_to(vat2[:, :, None], (128, 2, 128)).astype(FP8)
    ).reshape(128, 256)

    w['ones_f'] = np.ones((128, 1), f32)
    w['ident_f'] = np.eye(128, dtype=f32)
    w['ident_b'] = np.eye(128, dtype=BF16)
    w['ones_1x32'] = np.ones((1, NR), BF16)
    return w


_SHARED_KEYS = ('w1T', 'w2T', 'gmat_mu', 'gmat_cv1', 'gmat_cv2',
                'gn1_s', 'gn1_b', 'gn2_s',
                'gn2_b', 'wih_f', 'wih_b', 'whh_f', 'whh_b', 'lstm_bt_f',
                'lstm_bt_b', 'elT', 'el_b', 'et1', 'wc2T', 'comb_b',
                'gwihT', 'gwhhT', 'owT', 'out_b', 'vat8', 'ones_f',
                'ident_f', 'ident_b', 'ones_1x32')


def _per_core_maps(inp, w, nr=NR, ncores=NCORES):
    targets = np.asarray(inp['targets']).astype(np.int64)
    maps = []
    for core in range(ncores):
        n0 = core * nr
        m = {k: w[k] for k in _SHARED_KEYS}
        m['rois_cm'] = np.ascontiguousarray(w['rois_cm'][:, :, n0:n0 + nr])
        tm = np.zeros((VOC, NUMCH * nr), BF16)
        for s in range(NUMCH):
            for n in range(nr):
                tm[targets[n0 + n, s], s * nr + n] = 1.0
        m['tmask'] = tm
        oh0 = np.zeros((VOC, nr), BF16)
        oh0[0, :] = 1.0
        m['onehot0'] = oh0
        if w['has_gru_bias']:
            m['gru_bi'] = w['gru_bi']
            m['gru_bh'] = w['gru_bh']
        maps.append(m)
    return maps


# ----------------------------------------------------------------------------
# bass program
# ----------------------------------------------------------------------------

def _ap(tile, off, dims, pcount=None):
    """AP view of `tile`: keep partition dim (optionally limited to `pcount`
    partitions), free dims = [[step,count],...] at element offset `off`."""
    import concourse.bass as bass
    base = tile[:, :] if len(tile.shape) == 2 else tile
    p = list(base.ap[0])
    if pcount is not None:
        p = [p[0], pcount]
    return bass.AP(tensor=base.tensor, offset=base.offset + off,
                   ap=[p] + [list(d) for d in dims])


def _app(tile, pdim, off, dims):
    """AP with explicit partition dim [pstep, pcount]."""
    import concourse.bass as bass
    base = tile[:, :] if len(tile.shape) == 2 else tile
    return bass.AP(tensor=base.tensor, offset=base.offset + off,
                   ap=[list(pdim)] + [list(d) for d in dims])


def build_program(nr=NR, has_gru_bias=False, debug=False, phases=('conv','lstm','enc','dec')):
    _patch_tile_drain()
    import concourse.bass as bass
    import concourse.mybir as mybir
    from concourse.tile import TileContext

    dt = mybir.dt
    nc = bass.Bass(name="attp", trn_type="TRN2")

    D = {}
    def din(name, shape, dtype):
        D[name] = nc.dram_tensor(name, shape, dtype, kind="ExternalInput")

    din('rois_cm', [128, 2, nr, 17, 66], dt.float8e4)
    din('w1T', [128, 2 * 9 * 2 * 128], dt.float8e4)
    din('w2T', [128, 2 * 9 * 2 * 128], dt.float8e4)
    din('gmat_mu', [128, 128], dt.float32)
    din('gmat_cv1', [128, 128], dt.float32)
    din('gmat_cv2', [128, 128], dt.float32)
    for nm in ('gn1_s', 'gn1_b', 'gn2_s', 'gn2_b'):
        din(nm, [128, 2], dt.float32)
    for nm in ('wih_f', 'wih_b', 'whh_f', 'whh_b'):
        din(nm, [128, 2 * 8 * 128], dt.bfloat16)
    din('lstm_bt_f', [128, 8], dt.float32)
    din('lstm_bt_b', [128, 8], dt.float32)
    din('elT', [128, 4 * 2 * 128], dt.bfloat16)
    din('el_b', [128, 2], dt.float32)
    din('et1', [VOC, 256], dt.bfloat16)
    din('wc2T', [128, 2 * 2 * 128], dt.bfloat16)
    din('comb_b', [128, 2], dt.float32)
    din('gwihT', [128, 2 * 6 * 128], dt.bfloat16)
    din('gwhhT', [128, 2 * 6 * 128], dt.bfloat16)
    if has_gru_bias:
        din('gru_bi', [1, 768], dt.bfloat16)
        din('gru_bh', [1, 768], dt.bfloat16)
    din('owT', [128, 2 * VOC], dt.bfloat16)
    din('out_b', [VOC, 1], dt.float32)
    din('vat8', [128, 2 * 128], dt.float8e4)
    din('ones_f', [128, 1], dt.float32)
    din('ident_f', [128, 128], dt.float32)
    din('ident_b', [128, 128], dt.bfloat16)
    din('ones_1x32', [1, nr], dt.bfloat16)
    din('tmask', [VOC, NUMCH * nr], dt.bfloat16)
    din('onehot0', [VOC, nr], dt.bfloat16)

    out_parts = nc.dram_tensor('out_parts', [nr + VOC, 1], dt.float32,
                               kind="ExternalOutput")
    dbg = {}
    if debug:
        for nm, sh in (('seq', [2, 128, nr * T]),
                       ('hist', [2, 128, 2 * T * nr])):
            dbg[nm] = nc.dram_tensor('dbg_' + nm, sh, dt.float32,
                                     kind="ExternalOutput")

    with TileContext(nc) as tc:
        _body(nc, tc, D, out_parts, nr, has_gru_bias, dbg, mybir, phases)
    return nc


def _body(nc, tc, D, out_parts, nr, has_gru_bias, dbg, mybir, phases=('conv','lstm','enc','dec')):
    import contextlib
    dt = mybir.dt
    AF = mybir.ActivationFunctionType
    AL = mybir.AluOpType
    AX = mybir.AxisListType
    f32, bf16 = dt.float32, dt.bfloat16
    dma = nc.sync.dma_start
    TN = T * nr

    with contextlib.ExitStack() as ctx:
        consts = ctx.enter_context(tc.tile_pool(name="consts", bufs=1))

        def load(name):
            sh = list(D[name].shape)
            t = consts.tile(sh, D[name].dtype, tag=name)
            ix = tuple(slice(None) for _ in sh)
            dma(out=t, in_=D[name][ix])
            return t

        # conv-phase consts first so the roi DMAs aren't queued behind the
        # (large) lstm/decoder weight loads; w1T split across DMA queues so
        # the first conv matmul isn't gated on one serial transfer
        w1 = consts.tile([128, 2 * 9 * 2 * 128], dt.float8e4, tag='w1T',
                         name='w1T')
        for c4 in range(4):
            dma(out=w1[:, c4 * 1152:(c4 + 1) * 1152],
                in_=D['w1T'][:, c4 * 1152:(c4 + 1) * 1152])
        w2 = load('w2T')
        gm_mu = load('gmat_mu')
        gm_cv1 = load('gmat_cv1'); gm_cv2 = load('gmat_cv2')
        gn1s = load('gn1_s'); gn1b = load('gn1_b')
        gn2s = load('gn2_s'); gn2b = load('gn2_b')
        eps_t = consts.tile([128, 1], f32, tag="eps", name="eps")
        nc.vector.memset(eps_t, 1e-5)

        _rest = {}

        def load_rest():
            _rest['wih'] = [load('wih_f'), load('wih_b')]
            _rest['whh'] = [load('whh_f'), load('whh_b')]
            _rest['lstm_bt'] = [load('lstm_bt_f'), load('lstm_bt_b')]
            _rest['elT'] = load('elT'); _rest['el_b'] = load('el_b')
            _rest['et1'] = load('et1'); _rest['wc2T'] = load('wc2T')
            _rest['comb_b'] = load('comb_b')
            _rest['gwihT'] = load('gwihT'); _rest['gwhhT'] = load('gwhhT')
            if has_gru_bias:
                _rest['gru_bi'] = load('gru_bi')
                _rest['gru_bh'] = load('gru_bh')
                _rest['ones_n'] = load('ones_1x32')
            _rest['owT'] = load('owT'); _rest['out_b'] = load('out_b')
            _rest['vat8'] = load('vat8')
            _rest['ones_f'] = load('ones_f')
            _rest['ident_f'] = load('ident_f')
            _rest['ident_b'] = load('ident_b')
            _rest['tmask'] = load('tmask'); _rest['onehot0'] = load('onehot0')

        seq_p = ctx.enter_context(tc.tile_pool(name="seqp", bufs=1))
        seq = [seq_p.tile([128, TN], bf16, tag=f"seq{ct}", name=f"seq{ct}") for ct in range(2)]
        hist_p = ctx.enter_context(tc.tile_pool(name="histp", bufs=1))
        hist = [hist_p.tile([128, 2 * TN], bf16, tag=f"hist{d}", name=f"hist{d}")
                for d in range(2)]

        # ================= conv phase =================
        if 'conv' not in phases:
            for ct in range(2):
                nc.vector.memset(seq[ct], 0.01)
        if 'conv' in phases:
         DRM = mybir.MatmulPerfMode.DoubleRow
         f8 = dt.float8e4
         with tc.tile_pool(name="convp", bufs=2) as cp, \
             tc.tile_pool(name="convs", bufs=2) as cs, \
             tc.tile_pool(name="cpsum", bufs=2, space="PSUM") as cps, \
             tc.tile_pool(name="spsum", bufs=2, space="PSUM") as sps:
            GR = 2
            ngrp = nr // GR
            st = {}  # per-group pipeline state

            def s1_load_conv1(g):
                rr = [g * GR + r for r in range(GR)]
                pad1 = [cp.tile([128, 2 * 17 * 66], f8, tag=f"pad1_{r}",
                                name=f"pad1_{g}_{r}") for r in range(GR)]
                for r in range(GR):
                    dma(out=_ap(pad1[r], 0, [[1122, 2], [66, 17], [1, 66]]),
                        in_=D['rois_cm'][:, :, rr[r], :, :])
                stat1 = cs.tile([128, 24], f32, tag="stat1", name=f"st1_{g}")
                c1s = {}
                for ct in range(2):
                    c1 = [cps.tile([128, 512], f32, tag=f"c1_{r}",
                                   name=f"c1_{g}_{ct}_{r}") for r in range(GR)]
                    c1s[ct] = c1
                    for dy in range(3):
                        for dx in range(3):
                            k2 = (dy * 3 + dx) * 2 + ct
                            for r in range(GR):
                                rhs = _ap(pad1[r], dy * 66 + dx,
                                          [[1122, 2], [132, 8], [1, 64]])
                                nc.tensor.matmul(
                                    c1[r],
                                    _ap(w1, k2 * 256, [[128, 2], [1, 128]]),
                                    rhs, start=(dy + dx == 0),
                                    stop=(dy == 2 and dx == 2), perf_mode=DRM)
                    for r in range(GR):
                        j = (ct * GR + r) * 6
                        nc.vector.bn_stats(stat1[:, j:j + 6], c1[r])
                st[g] = {'c1s': c1s, 'stat1': stat1}

            def s2_gn1_relu(g):
                d = st[g]
                A1, B1 = _gn_stats_ab(nc, cs, sps, d['stat1'], gm_mu, gm_cv1,
                                      gn1s, gn1b, AF, AL, f32, eps_t, GR, '1')
                pad2 = [cp.tile([128, 2 * 9 * 66], f8, tag=f"pad2_{r}",
                                name=f"pad2_{g}_{r}") for r in range(GR)]
                for r in range(GR):
                    nc.gpsimd.memset(_ap(pad2[r], 0, [[594, 2], [1, 66]]), 0.0)
                    nc.gpsimd.memset(
                        _ap(pad2[r], 0, [[594, 2], [66, 9], [65, 2]]), 0.0)
                for ct in range(2):
                    for r in range(GR):
                        nc.scalar.activation(
                            out=_ap(pad2[r], ct * 594 + 67, [[66, 8], [1, 64]]),
                            in_=d['c1s'][ct][r], func=AF.Relu,
                            bias=B1[:, ct * GR + r:ct * GR + r + 1],
                            scale=A1[:, ct * GR + r:ct * GR + r + 1])
                d['pad2'] = pad2

            def s3_conv2(g):
                d = st[g]
                pad2 = d['pad2']
                stat2 = cs.tile([128, 24], f32, tag="stat2", name=f"st2_{g}")
                c2s = {}
                for ct in range(2):
                    c2t = cps.tile([128, 256 * GR], f32, tag="c2",
                                   name=f"c2_{g}_{ct}")
                    c2 = [_ap(c2t, r * 256, [[1, 256]]) for r in range(GR)]
                    c2s[ct] = c2
                    for dy in range(3):
                        for dx in range(3):
                            k2 = (dy * 3 + dx) * 2 + ct
                            for r in range(GR):
                                rhs = _ap(pad2[r], dy * 66 + dx,
                                          [[594, 2], [132, 4], [1, 64]])
                                nc.tensor.matmul(
                                    c2[r],
                                    _ap(w2, k2 * 256, [[128, 2], [1, 128]]),
                                    rhs, start=(dy + dx == 0),
                                    stop=(dy == 2 and dx == 2), perf_mode=DRM)
                    for r in range(GR):
                        j = (ct * GR + r) * 6
                        nc.vector.bn_stats(stat2[:, j:j + 6], c2[r])
                d['stat2'] = stat2
                d['c2s'] = c2s

            def s4_gn2_seq(g):
                d = st[g]
                A2, B2 = _gn_stats_ab(nc, cs, sps, d['stat2'], gm_mu, gm_cv2,
                                      gn2s, gn2b, AF, AL, f32, eps_t, GR, '2')
                for ct in range(2):
                    for r in range(GR):
                        c2n = cs.tile([128, 256], bf16, tag=f"c2n{ct}",
                                      name=f"c2n{g}_{ct}")
                        nc.scalar.activation(
                            out=c2n, in_=d['c2s'][ct][r], func=AF.Relu,
                            bias=B2[:, ct * GR + r:ct * GR + r + 1],
                            scale=A2[:, ct * GR + r:ct * GR + r + 1])
                        hs = cs.tile([128, 128], f32, tag=f"hs{ct}",
                                     name=f"hs{g}_{ct}")
                        nc.gpsimd.tensor_add(hs, c2n[:, 0:128],
                                             c2n[:, 128:256])
                        nc.gpsimd.tensor_add(
                            seq[ct][:, (g * GR + r) * T:(g * GR + r + 1) * T],
                            hs[:, 0:64], hs[:, 64:128])
                del st[g]

            # emission order per iteration: s4(g-3), s2(g-1), s1(g), s3(g-2)
            # keeps the gs-psum consumers early in the DVE stream so PE's
            # group matmuls never wait on a stale gs buffer
            stages = ((3, s4_gn2_seq), (1, s2_gn1_relu),
                      (0, s1_load_conv1), (2, s3_conv2))
            for k in range(ngrp + 3):
                for si, fn in stages:
                    g = k - si
                    if 0 <= g < ngrp:
                        fn(g)
                if k == 0:
                    load_rest()
        if not _rest:
            load_rest()
        wih = _rest['wih']; whh = _rest['whh']; lstm_bt = _rest['lstm_bt']
        elT = _rest['elT']; el_b = _rest['el_b']; et1 = _rest['et1']
        wc2T = _rest['wc2T']; comb_b = _rest['comb_b']
        gwihT = _rest['gwihT']; gwhhT = _rest['gwhhT']
        if has_gru_bias:
            gru_bi = _rest['gru_bi']; gru_bh = _rest['gru_bh']
            ones_n = _rest['ones_n']
        owT = _rest['owT']; out_b = _rest['out_b']; vat8 = _rest['vat8']
        ones_f = _rest['ones_f']; ident_f = _rest['ident_f']
        ident_b = _rest['ident_b']
        tmask = _rest['tmask']; onehot0 = _rest['onehot0']

        if dbg:
            for ct in range(2):
                tmp = seq_p.tile([128, TN], f32, tag=f"dbgs{ct}", name=f"dbgs{ct}")
                nc.vector.tensor_copy(tmp, seq[ct])
                dma(out=dbg['seq'][ct, :, :], in_=tmp)

        # ================= LSTM phase =================
        if 'lstm' not in phases:
            for d in range(2):
                nc.vector.memset(hist[d], 0.01)
        if 'lstm' in phases:
         with tc.tile_pool(name="xpp", bufs=1) as xp, \
             tc.tile_pool(name="lst", bufs=1) as lsp:
            # xproj[d]: [128, 8*TN], blocks [i0 i1 f0 f1 o0 o1 g0 g1], col t*nr+n
            colmap = {0: 0, 1: 1, 2: 2, 3: 3, 4: 6, 5: 7, 6: 4, 7: 5}
            xproj = [xp.tile([128, 8 * TN], bf16, tag=f"xp{d}", name=f"xp{d}")
                     for d in range(2)]
            with tc.tile_pool(name="xps", bufs=1, space="PSUM") as xps:
                nch = TN // 512
                tch = 512 // nr
                for d in range(2):
                    for mt in range(8):
                        pss = [xps.tile([128, 512], f32, tag=f"xpps{ch}",
                                        name=f"xpps{ch}")
                               for ch in range(nch)]
                        for kt in range(2):
                            for ch in range(nch):
                                rhs = _ap(seq[kt], ch * tch,
                                          [[1, tch], [64, nr]])
                                nc.tensor.matmul(
                                    pss[ch],
                                    wih[d][:, (kt * 8 + mt) * 128:
                                           (kt * 8 + mt) * 128 + 128],
                                    rhs, start=(kt == 0), stop=(kt == 1))
                        for ch in range(nch):
                            o0 = colmap[mt] * TN + ch * 512
                            if ch % 2 == 0:
                                nc.scalar.activation(
                                    out=xproj[d][:, o0:o0 + 512],
                                    in_=pss[ch], func=AF.Identity,
                                    bias=lstm_bt[d][:, mt:mt + 1])
                            else:
                                # split psum->sbuf moves across Act and DVE
                                nc.vector.tensor_scalar(
                                    out=xproj[d][:, o0:o0 + 512],
                                    in0=pss[ch],
                                    scalar1=lstm_bt[d][:, mt:mt + 1],
                                    scalar2=None, op0=AL.add)

            # h lives directly in hist; hzero holds the step-0 state
            hzero = lsp.tile([128, 2 * nr], bf16, tag="hzero", name="hzero")
            cst = [lsp.tile([128, 2 * nr], f32, tag=f"cst{d}", name=f"cst{d}")
                   for d in range(2)]
            nc.vector.memset(hzero, 0.0)
            for d in range(2):
                nc.vector.memset(cst[d], 0.0)

            with tc.tile_pool(name="gps", bufs=2, space="PSUM") as gpsp:
                for step in range(T):
                    for d in range(2):
                        t = step if d == 0 else T - 1 - step
                        gps = gpsp.tile([128, 8 * nr], f32, tag=f"g{d}", name=f"g{d}")
                        tprev = (t - 1) if d == 0 else (t + 1)
                        for mt in range(8):
                            cb = colmap[mt] * nr
                            for kt in range(2):
                                if step == 0:
                                    hrhs = hzero[:, kt * nr:(kt + 1) * nr]
                                else:
                                    hrhs = _ap(hist[d], kt * TN + tprev * nr,
                                               [[1, nr]])
                                nc.tensor.matmul(
                                    gps[:, cb:cb + nr],
                                    whh[d][:, (kt * 8 + mt) * 128:
                                           (kt * 8 + mt) * 128 + 128],
                                    hrhs,
                                    start=(kt == 0), stop=False,
                                    skip_group_check=True)
                        # fold x-projection in on the PE (frees the DVE add)
                        nc.tensor.matmul(
                            gps, ident_b,
                            _ap(xproj[d], t * nr, [[TN, 8], [1, nr]]),
                            start=False, stop=True, skip_group_check=True)
                        sgi = lsp.tile([128, 6 * nr], f32, tag=f"sgi{d}",
                                       name=f"sgi{d}")
                        tgg = lsp.tile([128, 2 * nr], f32, tag=f"tgg{d}",
                                       name=f"tgg{d}")
                        # i/f gates first so the c-state chain starts sooner
                        nc.scalar.activation(sgi[:, 0:4 * nr],
                                             gps[:, 0:4 * nr], AF.Sigmoid)
                        nc.scalar.activation(tgg, gps[:, 6 * nr:8 * nr],
                                             AF.Tanh)
                        nc.scalar.activation(sgi[:, 4 * nr:6 * nr],
                                             gps[:, 4 * nr:6 * nr],
                                             AF.Sigmoid)
                        tmp = lsp.tile([128, 2 * nr], f32, tag=f"tmp{d}", name=f"tmp{d}")
                        nc.vector.tensor_mul(tmp, sgi[:, 2 * nr:4 * nr], cst[d])
                        nc.vector.tensor_mul(tgg, sgi[:, 0:2 * nr], tgg)
                        nc.vector.tensor_add(cst[d], tmp, tgg)
                        tct = lsp.tile([128, 2 * nr], f32, tag=f"tct{d}", name=f"tct{d}")
                        nc.scalar.activation(tct, cst[d], AF.Tanh)
                        nc.vector.tensor_mul(
                            _ap(hist[d], t * nr, [[TN, 2], [1, nr]]),
                            sgi[:, 4 * nr:6 * nr], tct)

        if dbg:
            for d in range(2):
                tmp = hist_p.tile([128, 2 * TN], f32, tag=f"dbgh{d}", name=f"dbgh{d}")
                nc.vector.tensor_copy(tmp, hist[d])
                dma(out=dbg['hist'][d, :, :], in_=tmp)

        # ================= enc =================
        # enc_nt: [128, 2TN] ct-major, n-major inside (col ct*TN + n*T + t)
        # enc_tn: [128, 2TN] ct-major, t-major inside (col ct*TN + t*nr + n)
        enc_p = ctx.enter_context(tc.tile_pool(name="encp", bufs=1))
        enc_nt = enc_p.tile([128, 2 * TN], bf16, tag="ent", name="ent")
        enc_tn = enc_p.tile([128, 2 * TN], bf16, tag="etn", name="etn")
        with tc.tile_pool(name="eps", bufs=1, space="PSUM") as eps:
            NCH = TN // 512
            for ct in range(2):
                pss = [eps.tile([128, 512], f32, tag=f"encps{ch}",
                                name=f"encps{ch}")
                       for ch in range(NCH)]
                for kq in range(4):
                    d, kt = divmod(kq, 2)
                    for ch in range(NCH):
                        rhs = _ap(hist[d], kt * TN + ch * 8,
                                  [[1, 8], [nr, T]])
                        nc.tensor.matmul(
                            pss[ch], elT[:, (kq * 2 + ct) * 128:
                                         (kq * 2 + ct) * 128 + 128],
                            rhs, start=(kq == 0), stop=(kq == 3))
                for ch in range(NCH):
                    nc.vector.tensor_scalar(
                        out=enc_nt[:, ct * TN + ch * 512:
                                   ct * TN + ch * 512 + 512],
                        in0=pss[ch], scalar1=el_b[:, ct:ct + 1],
                        scalar2=None, op0=AL.add)
                    nc.scalar.activation(
                        out=_ap(enc_tn, ct * TN + ch * 8, [[1, 8], [nr, T]]),
                        in_=pss[ch], func=AF.Identity,
                        bias=el_b[:, ct:ct + 1])

        # ================= decoder =================
        # 2 independent roi streams; per step: fp8-DR scores with replicated
        # rows (no aw broadcast), softmax denominator folded after the
        # t-reduction, direct Sigmoid, partition_all_reduce argmax, deferred
        # log-sum-exp.
        from concourse import bass_isa
        DRM2 = mybir.MatmulPerfMode.DoubleRow
        f8 = dt.float8e4
        NS = 2
        NRS = nr // NS
        TNS = NRS * T
        dp = ctx.enter_context(tc.tile_pool(name="decp", bufs=2))
        accp = ctx.enter_context(tc.tile_pool(name="accp", bufs=1))
        sebuf = accp.tile([1, NUMCH * nr], f32, tag="sebuf", name="sebuf")
        acc_tgt = [accp.tile([VOC, 1], f32, tag=f"atg{s}", name=f"atg{s}")
                   for s in range(NS)]
        hid_bf = [accp.tile([128, 2 * NRS], bf16, tag=f"hb{s}", name=f"hb{s}")
                  for s in range(NS)]
        onehot = [accp.tile([VOC, NRS], bf16, tag=f"oh{s}", name=f"oh{s}")
                  for s in range(NS)]
        # logits live on all 128 partitions (pad rows at -1e30) so the
        # argmax partition_all_reduce can use power-of-two channels
        lsbt = [accp.tile([128, NRS], f32, tag=f"lsb{s}", name=f"lsb{s}")
                for s in range(NS)]
        for s in range(NS):
            nc.vector.memset(acc_tgt[s], 0.0)
            nc.vector.memset(hid_bf[s], 0.0)
            nc.vector.memset(lsbt[s], -1e30)
            nc.vector.tensor_copy(onehot[s],
                                  onehot0[:, s * NRS:(s + 1) * NRS])

        NUMCH_eff = NUMCH if 'dec' in phases else 0
        with tc.tile_pool(name="dpsA", bufs=1, space="PSUM") as dpsA, \
             tc.tile_pool(name="dpsC", bufs=2, space="PSUM") as dpsC, \
             tc.tile_pool(name="dpsG", bufs=1, space="PSUM") as dpsG:
            big = [dpsA.tile([128, TNS], f32, tag=f"big{s}", name=f"big{s}")
                   for s in range(NS)]
            _es = {}

            def dec_ph1(s, step):
              # chain-critical: schedule the attention front-end ASAP
              with tc.high_priority(offset=2000):
                # A = tanh(enc + hid), t-major, fp8 out for DR scores
                Aad = dp.tile([128, 2 * TNS], bf16, tag=f"Aad{s}",
                              name=f"Aad{s}")
                At = dp.tile([128, 2 * TNS], f8, tag=f"At{s}", name=f"At{s}")
                e = dp.tile([128, TNS], bf16, tag=f"e{s}", name=f"e{s}")
                # per 8-roi chunk j: add -> tanh -> scores -> exp flow
                # independently, so chunk j=1 overlaps chunk j=0's tail
                for j in range(NRS // 8):
                    for ct in range(2):
                        av = _ap(Aad, ct * TNS + j * 8, [[NRS, T], [1, 8]])
                        nc.vector.tensor_add(
                            av,
                            _ap(enc_tn, ct * TN + s * NRS + j * 8,
                                [[nr, T], [1, 8]]),
                            _ap(hid_bf[s], ct * NRS + j * 8,
                                [[0, T], [1, 8]]))
                        nc.scalar.activation(
                            _ap(At, ct * TNS + j * 8, [[NRS, T], [1, 8]]),
                            av, AF.Tanh)
                    rhs = _ap(At, j * 8, [[TNS, 2], [1, 8], [NRS, T]])
                    nc.tensor.matmul(big[s][:, j * 512:(j + 1) * 512],
                                     _ap(vat8, 0, [[128, 2], [1, 128]]),
                                     rhs, start=True, stop=True,
                                     perf_mode=DRM2)
                    nc.scalar.activation(e[:, j * 512:(j + 1) * 512],
                                         big[s][:, j * 512:(j + 1) * 512],
                                         AF.Exp, scale=1.0 / 16.0)
                # ctx numerator chunk follows its exp chunk immediately
                P = dp.tile([128, 2 * TNS], bf16, tag=f"P{s}", name=f"P{s}")
                for j in range(2):
                    nc.vector.tensor_mul(
                        _ap(P, j * 512, [[TNS, 2], [1, 512]]),
                        _ap(enc_nt, s * TNS + j * 512, [[TN, 2], [1, 512]]),
                        _ap(e, j * 512, [[0, 2], [1, 512]]))
                _es[s] = (e, P)

            def dec_ph2(s, step):
                e, P = _es[s]
                P2 = dp.tile([128, TNS], bf16, tag=f"P2{s}", name=f"P2{s}")
                P4 = dp.tile([128, TNS // 2], bf16, tag=f"P4{s}",
                             name=f"P4{s}")
                ctxr = dp.tile([128, 2 * NRS], f32, tag=f"cxr{s}",
                               name=f"cxr{s}")
                for j in range(2):
                    nc.vector.tensor_add(
                        _ap(P2, j * 256, [[TNS // 2, 2], [1, 256]]),
                        _ap(P, j * 512, [[TNS, 2], [T, 8], [1, 32]]),
                        _ap(P, j * 512 + 32, [[TNS, 2], [T, 8], [1, 32]]))
                    nc.vector.tensor_add(
                        _ap(P4, j * 128, [[TNS // 4, 2], [1, 128]]),
                        _ap(P2, j * 256, [[TNS // 2, 2], [32, 8], [1, 16]]),
                        _ap(P2, j * 256 + 16,
                            [[TNS // 2, 2], [32, 8], [1, 16]]))
                    nc.vector.tensor_reduce(
                        _ap(ctxr, j * 8, [[NRS, 2], [1, 8]]),
                        _ap(P4, j * 128, [[TNS // 4, 2], [16, 8], [1, 16]]),
                        axis=AX.X, op=AL.add)
                e2 = dp.tile([128, TNS // 2], bf16, tag=f"e2{s}",
                             name=f"e2{s}")
                nc.vector.tensor_add(e2, _ap(e, 0, [[T, NRS], [1, 32]]),
                                     _ap(e, 32, [[T, NRS], [1, 32]]))
                esum = dp.tile([128, NRS], f32, tag=f"es{s}", name=f"es{s}")
                nc.vector.tensor_reduce(
                    esum, _ap(e2, 0, [[32, NRS], [1, 32]]), axis=AX.X,
                    op=AL.add)
                rec = dp.tile([128, NRS], f32, tag=f"rc{s}", name=f"rc{s}")
                nc.vector.reciprocal(rec, esum)
                ctx_bf = dp.tile([128, 2 * NRS], bf16, tag=f"cxb{s}",
                                 name=f"cxb{s}")
                nc.vector.tensor_mul(ctx_bf, ctxr,
                                     _ap(rec, 0, [[0, 2], [1, NRS]]))
                # combine
                comb_bf = dp.tile([128, 2 * NRS], bf16, tag=f"cb{s}",
                                  name=f"cb{s}")
                for mt in range(2):
                    cpsd = dpsC.tile([128, NRS], f32, tag="small",
                                     name=f"cps{s}")
                    nc.tensor.matmul(cpsd, et1[:, mt * 128:mt * 128 + 128],
                                     onehot[s], start=True, stop=False)
                    for kt in range(2):
                        nc.tensor.matmul(
                            cpsd,
                            wc2T[:, (kt * 2 + mt) * 128:
                                 (kt * 2 + mt) * 128 + 128],
                            ctx_bf[:, kt * NRS:(kt + 1) * NRS],
                            start=False, stop=(kt == 1))
                    nc.scalar.activation(
                        out=comb_bf[:, mt * NRS:(mt + 1) * NRS],
                        in_=cpsd, func=AF.Relu, bias=comb_b[:, mt:mt + 1])
                # GRU: r,z input+hidden projections accumulate jointly
                gall = dpsG.tile([128, 8 * NRS], f32, tag=f"gal{s}",
                                 name=f"gal{s}")
                grz = gall[:, 0:4 * NRS]
                gin = gall[:, 4 * NRS:6 * NRS]
                ghn = gall[:, 6 * NRS:8 * NRS]
                nb = not has_gru_bias
                for mt in range(4):
                    oreg = grz[:, mt * NRS:(mt + 1) * NRS]
                    # hidden projections first: hid is ready long before comb
                    for kt in range(2):
                        nc.tensor.matmul(
                            oreg,
                            gwhhT[:, (kt * 6 + mt) * 128:
                                  (kt * 6 + mt) * 128 + 128],
                            hid_bf[s][:, kt * NRS:(kt + 1) * NRS],
                            start=(kt == 0), stop=False)
                    for kt in range(2):
                        nc.tensor.matmul(
                            oreg,
                            gwihT[:, (kt * 6 + mt) * 128:
                                  (kt * 6 + mt) * 128 + 128],
                            comb_bf[:, kt * NRS:(kt + 1) * NRS],
                            start=False, stop=(kt == 1 and nb))
                    if has_gru_bias:
                        nc.tensor.matmul(oreg,
                                         gru_bi[:, mt * 128:mt * 128 + 128],
                                         ones_n[:, 0:NRS],
                                         start=False, stop=False)
                        nc.tensor.matmul(oreg,
                                         gru_bh[:, mt * 128:mt * 128 + 128],
                                         ones_n[:, 0:NRS],
                                         start=False, stop=True)
                for mt in range(4, 6):
                    j = (mt - 4) * NRS
                    for kt in range(2):
                        nc.tensor.matmul(
                            gin[:, j:j + NRS],
                            gwihT[:, (kt * 6 + mt) * 128:
                                  (kt * 6 + mt) * 128 + 128],
                            comb_bf[:, kt * NRS:(kt + 1) * NRS],
                            start=(kt == 0), stop=(kt == 1 and nb))
                        nc.tensor.matmul(
                            ghn[:, j:j + NRS],
                            gwhhT[:, (kt * 6 + mt) * 128:
                                  (kt * 6 + mt) * 128 + 128],
                            hid_bf[s][:, kt * NRS:(kt + 1) * NRS],
                            start=(kt == 0), stop=(kt == 1 and nb))
                    if has_gru_bias:
                        nc.tensor.matmul(gin[:, j:j + NRS],
                                         gru_bi[:, mt * 128:mt * 128 + 128],
                                         ones_n[:, 0:NRS],
                                         start=False, stop=True)
                        nc.tensor.matmul(ghn[:, j:j + NRS],
                                         gru_bh[:, mt * 128:mt * 128 + 128],
                                         ones_n[:, 0:NRS],
                                         start=False, stop=True)
                with tc.high_priority(offset=1000):
                    rz = dp.tile([128, 4 * NRS], f32, tag=f"rz{s}",
                                 name=f"rz{s}")
                    nc.scalar.activation(rz, grz, AF.Sigmoid)
                    t1 = dp.tile([128, 2 * NRS], f32, tag=f"t1{s}",
                                 name=f"t1{s}")
                    nc.vector.tensor_mul(t1, rz[:, 0:2 * NRS], ghn)
                    nnt = dp.tile([128, 2 * NRS], f32, tag=f"nt{s}",
                                  name=f"nt{s}")
                    nc.vector.scalar_tensor_tensor(
                        out=nnt, in0=gin, scalar=0.0, in1=t1,
                        op0=AL.bypass, op1=AL.add)
                    nc.scalar.activation(nnt, nnt, AF.Tanh)
                    dd = dp.tile([128, 2 * NRS], f32, tag=f"dd{s}",
                                 name=f"dd{s}")
                    nc.vector.tensor_sub(dd, hid_bf[s], nnt)
                    nc.vector.tensor_mul(dd, rz[:, 2 * NRS:4 * NRS], dd)
                    nc.vector.tensor_add(hid_bf[s], nnt, dd)

            def dec_ph3(s, step):
                # logits + loss + argmax-onehot (off the critical path)
                lg = dpsC.tile([VOC, NRS], f32, tag="small", name=f"lg{s}")
                for kt in range(2):
                    nc.tensor.matmul(lg, owT[:, kt * VOC:(kt + 1) * VOC],
                                     hid_bf[s][:, kt * NRS:(kt + 1) * NRS],
                                     start=(kt == 0), stop=(kt == 1))
                lsb = lsbt[s][0:VOC, :]
                nc.scalar.activation(lsb, lg, AF.Identity, bias=out_b[:, 0:1])
                if step < NUMCH - 1:
                    lgT_ps = dpsC.tile([NRS, VOC], f32, tag="small",
                                       name=f"lgT{s}")
                    nc.tensor.transpose(lgT_ps, lsb, ident_f[0:VOC, 0:VOC])
                    lgT = dp.tile([NRS, VOC], f32, tag=f"lgT{s}",
                                  name=f"lgTs{s}")
                    nc.vector.tensor_copy(lgT, lgT_ps)
                    mx8 = dp.tile([NRS, 8], f32, tag=f"mx{s}", name=f"mx{s}")
                    nc.vector.max(out=mx8, in_=lgT)
                    mT = dp.tile([NRS, VOC], f32, tag=f"mT{s}", name=f"mT{s}")
                    nc.vector.tensor_scalar(out=mT, in0=lgT,
                                            scalar1=mx8[:, 0:1], scalar2=None,
                                            op0=AL.is_equal)
                    oh_ps = dpsC.tile([VOC, NRS], f32, tag="small",
                                      name=f"ohp{s}")
                    nc.tensor.transpose(oh_ps, mT, ident_f[0:NRS, 0:NRS])
                    nc.vector.tensor_copy(onehot[s], oh_ps)
                ex = dp.tile([VOC, NRS], f32, tag=f"ex{s}", name=f"ex{s}")
                nc.scalar.activation(ex, lsb, AF.Exp)
                se_ps = dpsC.tile([1, NRS], f32, tag="small", name=f"se{s}")
                nc.tensor.matmul(se_ps, ones_f[0:VOC, 0:1], ex,
                                 start=True, stop=True)
                nc.vector.tensor_copy(
                    sebuf[:, step * nr + s * NRS:step * nr + s * NRS + NRS],
                    se_ps)
                junk = dp.tile([VOC, NRS], f32, tag=f"jk{s}", name=f"jk{s}")
                ttmp = dp.tile([VOC, 1], f32, tag=f"tt{s}", name=f"tt{s}")
                nc.vector.scalar_tensor_tensor(
                    out=junk, in0=lsb, scalar=0.0,
                    in1=tmask[:, step * nr + s * NRS:
                              step * nr + s * NRS + NRS],
                    op0=AL.bypass, op1=AL.mult, accum_out=ttmp)
                nc.gpsimd.tensor_add(acc_tgt[s], acc_tgt[s], ttmp)

            # stream-interleaved emission: while stream s waits on its
            # attention chain (tanh->scores->exp), the other stream's
            # vector work keeps DVE's in-order queue busy; the logits/loss
            # block (ph3) is deferred to the iteration tail so it never
            # delays the next tanh/exp in the Act queue
            for step in range(NUMCH_eff):
                dec_ph1(0, step)
                if step > 0:
                    dec_ph2(1, step - 1)
                dec_ph1(1, step)
                dec_ph2(0, step)
                if step > 0:
                    dec_ph3(1, step - 1)
                dec_ph3(0, step)
            if NUMCH_eff:
                dec_ph2(1, NUMCH_eff - 1)
                dec_ph3(1, NUMCH_eff - 1)
        if 'dec' not in phases:
            nc.vector.memset(sebuf, 1.0)
        lse_ln = accp.tile([1, NUMCH * nr], f32, tag="lse_ln", name="lse_ln")
        nc.scalar.activation(lse_ln, sebuf, AF.Ln)
        acc_lse = accp.tile([1, nr], f32, tag="acc_lse", name="acc_lse")
        nc.vector.tensor_reduce(
            acc_lse, _ap(lse_ln, 0, [[1, nr], [nr, NUMCH]]),
            axis=AX.X, op=AL.add)
        nc.vector.tensor_add(acc_tgt[0], acc_tgt[0], acc_tgt[1])
        dma(out=out_parts[0:nr, :], in_=acc_lse)
        dma(out=out_parts[nr:nr + VOC, :], in_=acc_tgt[0])


def _gn_stats_ab(nc, pool, psum_pool, stat, gm_mu, gm_cv, gn_s, gn_b,
                 AF, AL, f32, eps_t, R, tagsfx):
    """GroupNorm A/B from bn_stats outputs for both cts of one conv layer.

    stat: [128, 12R] = per (ct,r) the 6 bn_stats cols
    (cnt_e, mean_e, cnt_e*var_e, cnt_o, mean_o, cnt_o*var_o).
    Returns A, B [128, 2R] ((ct,r)-major): A = s*rstd, B = b - mu*A.
    """
    M = 2 * R
    msq = pool.tile([128, 2 * M], f32, tag="msq" + tagsfx, name="msq" + tagsfx)
    mv = _ap(stat, 1, [[6, M], [3, 2]])
    nc.vector.tensor_mul(msq, mv, mv)
    gs = psum_pool.tile([128, 2 * M], f32, tag="gs", name="gs" + tagsfx)
    mu_ap = _ap(gs, 0, [[1, M]])
    e2_ap = _ap(gs, M, [[1, M]])
    # mu = sum_p (mean_e + mean_o)/16
    nc.tensor.matmul(mu_ap, gm_mu, _ap(stat, 1, [[6, M]]), start=True,
                     stop=False)
    nc.tensor.matmul(mu_ap, gm_mu, _ap(stat, 4, [[6, M]]), start=False,
                     stop=True)
    # E2 = sum_p [(cv_e+cv_o)/(8 cnt) + (msq_e+msq_o)/16]
    nc.tensor.matmul(e2_ap, gm_cv, _ap(stat, 2, [[6, M]]), start=True,
                     stop=False)
    nc.tensor.matmul(e2_ap, gm_cv, _ap(stat, 5, [[6, M]]), start=False,
                     stop=False)
    nc.tensor.matmul(e2_ap, gm_mu, _ap(msq, 0, [[2, M]]), start=False,
                     stop=False)
    nc.tensor.matmul(e2_ap, gm_mu, _ap(msq, 1, [[2, M]]), start=False,
                     stop=True)
    A = pool.tile([128, M], f32, tag="gnA" + tagsfx, name="gnA" + tagsfx)
    B = pool.tile([128, M], f32, tag="gnB" + tagsfx, name="gnB" + tagsfx)
    muE = pool.tile([128, 2 * M], f32, tag="gnm" + tagsfx, name="gnm" + tagsfx)
    var = pool.tile([128, M], f32, tag="gnv" + tagsfx, name="gnv" + tagsfx)
    nc.vector.tensor_copy(muE, gs)
    mu = muE[:, 0:M]
    e2 = muE[:, M:2 * M]
    nc.vector.tensor_mul(var, mu, mu)
    nc.vector.scalar_tensor_tensor(out=var, in0=e2, scalar=0.0, in1=var,
                                   op0=AL.bypass, op1=AL.subtract)
    nc.scalar.activation(var, var, AF.Sqrt, bias=eps_t[:, 0:1])
    nc.vector.reciprocal(var, var)
    # A = rstd * s  (s broadcast per ct across the R rois)
    nc.vector.tensor_mul(A, var, _ap(gn_s, 0, [[1, 2], [0, R]]))
    nc.vector.tensor_mul(var, mu, A)
    # B = b - mu*A
    nc.vector.scalar_tensor_tensor(
        out=B, in0=_ap(gn_b, 0, [[1, 2], [0, R]]), scalar=0.0, in1=var,
        op0=AL.bypass, op1=AL.subtract)
    return A, B


def _gn_ab4(nc, pool, gs, s_col, b_col, AF, AL, f32, eps_t, R):
    """gs: psum [128, 2R] = [mu_r0, E2_r0, ...] for one ct across R rois.
    Returns (A, B) tiles [128, R]: A = rstd*s, B = b - mu*A."""
    A = pool.tile([128, R], f32, tag="gnA", name="gnA")
    B = pool.tile([128, R], f32, tag="gnB", name="gnB")
    mu = pool.tile([128, R], f32, tag="gnmu", name="gnmu")
    var = pool.tile([128, R], f32, tag="gnvar", name="gnvar")
    nc.vector.tensor_copy(mu, _ap(gs, 0, [[2, R]]))
    nc.vector.tensor_mul(var, mu, mu)
    nc.vector.scalar_tensor_tensor(
        out=var, in0=_ap(gs, 1, [[2, R]]), scalar=0.0, in1=var,
        op0=AL.bypass, op1=AL.subtract)
    nc.scalar.activation(var, var, AF.Sqrt, bias=eps_t[:, 0:1])
    nc.vector.reciprocal(var, var)
    nc.vector.tensor_scalar_mul(A, var, s_col)
    nc.vector.tensor_mul(mu, mu, A)
    nc.vector.tensor_scalar(out=B, in0=mu, scalar1=b_col, scalar2=-1.0,
                            op0=AL.subtract, op1=AL.mult)
    return A, B


def _gn_ab(nc, pool, gs, s_const, b_const, AF, AL, f32, eps_t):
    """gs psum [128,4] = [mu0, E2_0, mu1, E2_1] -> AB [128,4] = [A0,A1,B0,B1]:
    A = rstd*s, B = b - mu*A."""
    AB = pool.tile([128, 4], f32, tag="AB", name="AB")
    mu = pool.tile([128, 2], f32, tag="gnmu", name="gnmu")
    var = pool.tile([128, 2], f32, tag="gnvar", name="gnvar")
    nc.vector.tensor_copy(mu, _ap(gs, 0, [[2, 2]]))
    nc.vector.tensor_mul(var, mu, mu)
    nc.vector.scalar_tensor_tensor(
        out=var, in0=_ap(gs, 1, [[2, 2]]), scalar=0.0, in1=var,
        op0=AL.bypass, op1=AL.subtract)
    nc.scalar.activation(var, var, AF.Sqrt, bias=eps_t[:, 0:1])
    nc.vector.reciprocal(var, var)
    nc.vector.tensor_mul(AB[:, 0:2], var, s_const)
    nc.vector.tensor_mul(mu, mu, AB[:, 0:2])
    nc.vector.scalar_tensor_tensor(
        out=AB[:, 2:4], in0=b_const, scalar=0.0, in1=mu,
        op0=AL.bypass, op1=AL.subtract)
    return AB


# ----------------------------------------------------------------------------
# driver
# ----------------------------------------------------------------------------

def kernel(**inputs):
    w = _prep_weights(inputs)
    maps = _per_core_maps(inputs, w)
    key = ('v2', w['has_gru_bias'])
    if key not in _PROG_CACHE:
        nc = build_program(NR, w['has_gru_bias'], debug=False)
        dedupe_ldweights(nc)
        split_excess_waits(nc)
        _PROG_CACHE[key] = nc
    nc = _PROG_CACHE[key]
    from concourse import bass_utils
    r = bass_utils.run_bass_kernel_spmd(nc, maps, core_ids=list(range(NCORES)))
    tot = 0.0
    for core in range(NCORES):
        parts = r.results[core]['out_parts'][:, 0].astype(np.float64)
        tot += parts[:NR].sum() - parts[NR:].sum()
    return np.float32(tot / N_FULL)


